# revision 63
# baseline (speedup 1.0000x reference)
"""Multi-head causal attention on 8 Trainium2 NeuronCores.

Problem: x [2, 2048, 1024] f32; Wq/Wk/Wv [1024, 1024]; Wo [1024, 1024]; bo [1024].
  q/k/v = split_heads(x @ W*)  (16 heads, head_dim 64)
  scores = q k^T, causal mask, / sqrt(1024), softmax, out = (w v) @ Wo + bo

Sharding: tensor-parallel over heads, 8-way (Megatron-style): core c computes
heads {2c, 2c+1} for BOTH batches. The concat+out_proj needs all heads, so
cores exchange attention outputs with 8-rank AllToAlls. Restructure vs the
296us baseline (whose tail after attention was ~75us: exposed A2As + a
zero-padded double out_proj at half clock) -- ~278us typical, best 263us:

 - per-(head,batch) AllToAlls triggered at the 25/50/75/100% marks of the
   attention pipeline (block order h0b0, h1b0, h0b1, h1b1). Triggered
   on-time and so 8-core-synchronized, each 256KB exchange completes in
   ~6us (vs ~30us when skewed). The LAST exchange (h1,b1) is further split
   into two 128KB halves (batch-1 tokens interleaved per-slot so each half
   covers all 8 destinations): half A + most out_proj work overlap the
   pipeline; only a ~6us exchange + one 4K-cycle pass trail it.
 - out_proj split into per-exchange partial passes, run as PE filler work
   inside the attention pipeline as each A2A lands.
 - out_proj contraction fully packed: the gathered 64-row head chunks are
   paired into 128-partition stationaries (chunk 2p in partitions 0:64,
   chunk 2p+1 in 64:128) against a row-gathered Wo ("wo_pack"), so out_proj
   costs 32768 PE cycles total instead of 65536 zero-padded ones.
 - output resharded: core j returns batch-0 tokens [256j,256j+256) and
   batch-1 tokens [128j,+128) and [1024+128j,+128); host reassembles.
 - A2A staging rides the sync queue (kept free of bulk loads: b1 x loads
   dispatch from the scalar queue), collective triggers ALWAYS precede
   gathers on the gpsimd queue, and the final block's causal masks run on
   DVE -- all so no collective-completion wait ever delays a trigger, a
   mask, or staging (each such coupling measured 30-70us).

Measured dead ends kept out: fp8 DoubleRow for proj/AV (this problem's
attention output is a near-uniform mean of ~random values, so the signal
averages down as fast as fp8 noise: 2-4% rel err vs the 2% budget); DMA
xbar transpose for x^T (serializes against collectives, destroying A2A
overlap); gpsimd casts (4x slower than DVE); --enable-ldw-opt (rejects
is_transpose/1-partition LDWEIGHTS).

On-chip layout trick (unchanged): attention is computed fully transposed
(scores^T = K Q^T in [k, q] layout) so the softmax weights come out exactly in
the layout the attn-value matmul wants as its moving operand, and the AV
result comes out as attn^T [d, q] which is exactly the stationary layout
out_proj wants. The softmax denominator is obtained for free by augmenting V
with a ones-column (row 64 of the AV psum accumulates sum(w)).

Compute dtype bf16 (fp32 accumulation in PSUM).
"""

from contextlib import ExitStack

import numpy as np

import concourse.bass as bass
import concourse.tile as tile
from concourse import bacc, mybir
from concourse.bass_utils import run_bass_kernel_spmd
from concourse.masks import make_identity

F32 = mybir.dt.float32
BF16 = mybir.dt.bfloat16

N_CORES = 8
B = 2
S = 2048
D = 1024
H = 16
DH = 64
H_PER = 2              # heads per core
DCOL = H_PER * DH      # 128: projection output cols per core
KT = D // 128          # 8 contraction tiles
SB = S // 128          # 16 sequence blocks
NQS = S // 512         # 4 q-spans
TOK = S // 8           # 256 output tokens per core per batch
SCALE = 1.0 / np.sqrt(np.float32(D))

_CACHE = {}


def build():
    """Build the SPMD program (identical on all 8 cores)."""
    nc = bacc.Bacc("TRN2", target_bir_lowering=False, debug=False)

    x_t = nc.dram_tensor("x", [B, S, D], F32, kind="ExternalInput")
    wq_t = nc.dram_tensor("wq", [D, DCOL], F32, kind="ExternalInput")
    wk_t = nc.dram_tensor("wk", [D, DCOL], F32, kind="ExternalInput")
    wv_t = nc.dram_tensor("wv", [D, DCOL], F32, kind="ExternalInput")
    wo_t = nc.dram_tensor("wo", [D, D], F32, kind="ExternalInput")
    bo_t = nc.dram_tensor("bo", [1, D], F32, kind="ExternalInput")
    # rows 0:TOK = batch-0 tokens [256c, 256c+256), rows TOK:2TOK = batch 1
    out_t = nc.dram_tensor("out", [B * TOK, D], F32, kind="ExternalOutput")

    # collective buffers (internal DRAM): one A2A per (local head, batch).
    # slot j of a2a_in goes to rank j = that head's attn^T restricted to
    # tokens [256j, 256j+256) of that batch; slot c of a2a_out arrived from
    # rank c = global head 2c+h for MY 256-token slice.
    warm_in = nc.dram_tensor("warm_in", [8, 16], F32)
    warm_out = nc.dram_tensor("warm_out", [8, 16], F32)
    a2a_in = {}
    a2a_out = {}
    for h in range(H_PER):
        for b in range(B):
            a2a_in[h, b] = nc.dram_tensor(f"a2a_in{h}{b}", [8, DH, TOK], BF16)
            a2a_out[h, b] = nc.dram_tensor(f"a2a_out{h}{b}", [8, DH, TOK], BF16)
    # the LAST exchange (h1, b1) is split into two 128KB halves so the first
    # half (+ its out_proj work) overlaps the pipeline tail; batch-1 tokens
    # are interleaved per-slot (slot j: tokens [128j,128j+128) of each
    # 1024-token half) so each half covers all 8 destinations.
    a2a_in11 = [nc.dram_tensor(f"a2a_in11{x}", [8, DH, 128], BF16)
                for x in "ab"]
    a2a_out11 = [nc.dram_tensor(f"a2a_out11{x}", [8, DH, 128], BF16)
                 for x in "ab"]

    with tile.TileContext(nc) as tc, ExitStack() as ctx:
        const = ctx.enter_context(tc.tile_pool(name="const", bufs=1))
        persist = ctx.enter_context(tc.tile_pool(name="persist", bufs=1))
        stage = ctx.enter_context(tc.tile_pool(name="stage", bufs=2))
        wstage = ctx.enter_context(tc.tile_pool(name="wstage", bufs=1))
        wpool = ctx.enter_context(tc.tile_pool(name="wpool", bufs=6))
        rbpool = ctx.enter_context(tc.tile_pool(name="rbpool", bufs=2))
        spool = ctx.enter_context(tc.tile_pool(name="spool", bufs=2))
        opool = ctx.enter_context(tc.tile_pool(name="opool", bufs=2))
        ps_mm = ctx.enter_context(tc.tile_pool(name="ps_mm", bufs=2, space="PSUM"))
        ps_o = ctx.enter_context(tc.tile_pool(name="ps_o", bufs=2, space="PSUM"))
        ps_x = ctx.enter_context(tc.tile_pool(name="ps_x", bufs=2, space="PSUM"))

        # ---- warmup collective: absorbs the per-execution ncfw entry cost
        # concurrently with the compute phase.
        nc.gpsimd.collective_compute(
            "AllToAll", mybir.AluOpType.bypass,
            replica_groups=[list(range(8))],
            ins=[warm_in.ap().opt()], outs=[warm_out.ap().opt()],
        )

        identity = const.tile([128, 128], BF16)
        make_identity(nc, identity)
        ones1 = const.tile([1, DH], BF16)
        nc.vector.memset(ones1, 1.0)
        # 4 causal masks (mask[r][p,f] = 1 iff f - p - 128r >= 0) for the
        # final attention block, whose masking runs on DVE instead of gpsimd
        # so the gpsimd queue (collective triggers + gathers) is never a
        # masking dependency at the tail.
        masks = []
        for r in range(4):
            mk = const.tile([128, 512], BF16, name=f"mask{r}")
            nc.vector.memset(mk, 1.0)
            nc.gpsimd.affine_select(
                out=mk, in_=mk,
                pattern=[[1, 512]],
                compare_op=mybir.AluOpType.is_ge,
                fill=0.0,
                base=-128 * r,
                channel_multiplier=-1)
            masks.append(mk)

        attnT = [
            persist.tile([128, S], BF16, tag=f"attnT{b}", name=f"attnT{b}")
            for b in range(B)
        ]

        # ---- phase helpers -------------------------------------------------
        def transpose_x_step(b, xT, sb):
            """One 128-row block of x[b] -> x^T bf16 columns, via PE
            transpose. Batch 0's cast runs on the otherwise-idle ACT."""
            xn = stage.tile([128, D], F32, tag="xn", name="xn", bufs=4)
            # b1's bulk loads dispatch from the scalar queue so the sync
            # queue (A2A staging) never backs up behind 8MB of x traffic;
            # their buffer-free waits resolve via DVE casts, which never
            # depend on later scalar-queue work (no cross-queue cycle).
            (nc.sync if b == 0 else nc.scalar).dma_start(
                out=xn, in_=x_t[b, sb * 128:(sb + 1) * 128, :])
            xb = stage.tile([128, D], BF16, tag=f"xb{b}", name="xb", bufs=2)
            # b0's cast runs on the otherwise-idle ACT; b1's on DVE (gpsimd
            # casts measured 4x slower, and ACT is exp-critical by then).
            if b == 0:
                nc.scalar.copy(xb, xn)
            else:
                nc.vector.tensor_copy(out=xb, in_=xn)
            for g in range(2):  # 2 groups of 4 d-blocks -> one psum bank
                pt = ps_x.tile([128, 4, 128], BF16, tag="x", name="pt")
                for k in range(4):
                    kt = g * 4 + k
                    nc.tensor.transpose(
                        pt[:, k, :], xb[:, kt * 128:(kt + 1) * 128], identity)
                dst = xT[:, g * 4:(g + 1) * 4, sb * 128:(sb + 1) * 128]
                nc.vector.tensor_copy(out=dst, in_=pt)

        def load_weights():
            def load_cast(dram_ap, kt_cols, name):
                st = wstage.tile([128, KT, kt_cols], F32, tag="wst",
                                 name="wst")
                nc.sync.dma_start(
                    out=st, in_=dram_ap.rearrange("(kt p) c -> p kt c", p=128))
                bf = persist.tile([128, KT, kt_cols], BF16, tag=name, name=name)
                nc.vector.tensor_copy(out=bf, in_=st)
                return bf

            wq_bf = load_cast(wq_t[:, :], DCOL, "wq")
            wk_bf = load_cast(wk_t[:, :], DCOL, "wk")
            wv_bf = load_cast(wv_t[:, :], DCOL, "wv")
            return wq_bf, wk_bf, wv_bf

        def alloc_proj_tiles(b):
            # Q^T / K^T are stored per-head, zero-padded to 128 partitions
            # (rows 64:128 = 0) because matmuls with contraction dim <= 64
            # run at HALF rate on the PE.
            qT = [persist.tile([128, S], BF16, tag=f"qT{b}{h}",
                               name=f"qT{b}{h}") for h in range(H_PER)]
            kTt = [persist.tile([128, S], BF16, tag=f"kT{b}{h}",
                                name=f"kT{b}{h}") for h in range(H_PER)]
            vp = persist.tile([128, SB, H_PER * (DH + 1)], BF16,
                              tag=f"vp{b}", name=f"vp{b}")
            for t in qT + kTt:
                nc.gpsimd.memset(t[DH:128, :], 0.0)
            return qT, kTt, vp

        def proj_qk_step(xT, w_bf, dest, nt):
            ps = ps_x.tile([128, 512], F32, tag="x", name="ps")
            for kt in range(KT):
                nc.tensor.matmul(
                    ps, lhsT=w_bf[:, kt, :],
                    rhs=xT[:, kt, nt * 512:(nt + 1) * 512],
                    start=(kt == 0), stop=(kt == KT - 1))
            for h in range(H_PER):
                nc.vector.tensor_copy(
                    out=dest[h][0:DH, nt * 512:(nt + 1) * 512],
                    in_=ps[h * DH:(h + 1) * DH, :])

        def proj_v_step(xT, wv_bf, vp, sb):
            ps = ps_x.tile([128, 512], F32, tag="x", name="ps")
            for kt in range(KT):
                nc.tensor.matmul(
                    ps[:, 0:DCOL], lhsT=xT[:, kt, sb * 128:(sb + 1) * 128],
                    rhs=wv_bf[:, kt, :],
                    start=(kt == 0), stop=(kt == KT - 1))
            dst = vp.rearrange("p s (h c) -> p s h c", c=DH + 1)[:, sb, :, :DH]
            nc.vector.tensor_copy(
                out=dst, in_=ps[:, 0:DCOL].rearrange("p (h c) -> p h c", c=DH))

        # Attention, software-pipelined GLOBALLY across (batch, head, q-span)
        # in PAIRS of k-blocks: scores for a pair land in a 2-bank psum tile,
        # ONE exp covers both halves, and the AV matmuls trail DEPTH pairs
        # behind, so the PE stream has a single priming point for the whole
        # attention phase.
        DEPTH = 4

        def normalize_evict(b, h, qs, o_ps):
            hr = h * DH
            denom = spool.tile([1, 512], F32, tag="den", name="denom")
            nc.vector.tensor_copy(out=denom, in_=o_ps[DH:DH + 1, :])
            recip_f = spool.tile([1, 512], F32, tag="recf", name="recip_f")
            nc.vector.reciprocal_approx_fast(out=recip_f, in_=denom)
            recip = spool.tile([1, 512], BF16, tag="rec", name="recip")
            nc.vector.tensor_copy(out=recip, in_=recip_f)
            # broadcast recip across the 64 partitions through the PE array
            # (the only cross-partition fabric that isn't a DMA round-trip).
            rb_ps = ps_x.tile([DH, 512], F32, tag="x", name="rb_ps")
            nc.tensor.matmul(rb_ps, lhsT=ones1, rhs=recip,
                             start=True, stop=True)
            rb = rbpool.tile([DH, 512], F32, tag="rb", name="rb")
            nc.vector.tensor_copy(out=rb, in_=rb_ps)
            dst = attnT[b][hr:hr + DH, qs * 512:(qs + 1) * 512]
            nc.vector.tensor_mul(dst, o_ps[0:DH, :], rb)
            # stage this span's attn^T to the A2A input right away. b0:
            # slot j = contiguous 256-token slice j (span covers slots 2qs,
            # 2qs+1). b1: interleaved map -- token t<1024 goes to slot
            # t//128 col t%128 (half A), t>=1024 to slot (t-1024)//128 col
            # 128+ (half B) -- so each half covers all 8 destinations.
            if b == 0:
                for j in range(2):
                    nc.sync.dma_start(
                        out=a2a_in[h, b][2 * qs + j],
                        in_=dst[:, j * TOK:(j + 1) * TOK])
            else:
                half, base = qs // 2, 4 * (qs % 2)
                for i in range(4):
                    if h == 0:
                        out_ap = a2a_in[0, 1][base + i, :,
                                              half * 128:(half + 1) * 128]
                    else:
                        out_ap = a2a_in11[half][base + i]
                    nc.sync.dma_start(
                        out=out_ap, in_=dst[:, i * 128:(i + 1) * 128])

        def attention_steps(h, b, qT, kTt, vp):
            """Yield (emit_scores, emit_av) closures, one pair per k-block,
            processing TWO q-spans at once (same stationary operand ->
            LDWEIGHTS dedups between the two matmuls). Epilogues fire from
            the AV closure that completes each span."""
            qTh, kTh = qT[h], kTt[h]
            vslice = vp[:, :, h * (DH + 1):(h + 1) * (DH + 1)]
            for qp in range(NQS // 2):
                qsA, qsB = 2 * qp, 2 * qp + 1
                lastA, lastB = 4 * qsA + 3, 4 * qsB + 3
                o_A = ps_o.tile([DH + 1, 512], F32, tag="o", name="o_A")
                o_B = ps_o.tile([DH + 1, 512], F32, tag="o", name="o_B")
                box = {}

                def mk_scores(kb, qsA=qsA, qsB=qsB, lastA=lastA, box=box):
                    def emit_scores():
                        s_ps = ps_mm.tile([128, 2, 512], F32, tag="mm",
                                          name="s_ps")
                        spans = ([(0, qsA)] if kb <= lastA else []) + [(1, qsB)]
                        for i, qs in spans:
                            nc.tensor.matmul(
                                s_ps[:, i, :],
                                lhsT=kTh[:, kb * 128:(kb + 1) * 128],
                                rhs=qTh[:, qs * 512:(qs + 1) * 512],
                                start=True, stop=True)
                        w_bf_t = wpool.tile([128, 2, 512], BF16, tag="w",
                                            name="w_bf_t")
                        if kb < 4 * qsA and len(spans) == 2:
                            # clean interior for both spans: one big exp
                            nc.scalar.activation(
                                w_bf_t, s_ps, mybir.ActivationFunctionType.Exp,
                                scale=float(SCALE))
                        else:
                            for i, qs in spans:
                                lo = max(0, 128 * (kb - 4 * qs))
                                nc.scalar.activation(
                                    w_bf_t[:, i, lo:512], s_ps[:, i, lo:512],
                                    mybir.ActivationFunctionType.Exp,
                                    scale=float(SCALE))
                        for i, qs in spans:
                            if kb >= 4 * qs:
                                # causal: keep iff (512qs+f) - (128kb+p) >= 0.
                                # Final block masks on DVE so the gpsimd
                                # queue (gathers + triggers) is never a
                                # masking dependency; the stale [0:lo] region
                                # is finite (prior exp outputs) so mul-by-0
                                # safely zeroes it.
                                if h == 1 and b == 1:
                                    nc.vector.tensor_mul(
                                        w_bf_t[:, i, :], w_bf_t[:, i, :],
                                        masks[kb - 4 * qs])
                                else:
                                    nc.gpsimd.affine_select(
                                        out=w_bf_t[:, i, :],
                                        in_=w_bf_t[:, i, :],
                                        pattern=[[1, 512]],
                                        compare_op=mybir.AluOpType.is_ge,
                                        fill=0.0,
                                        base=512 * qs - 128 * kb,
                                        channel_multiplier=-1)
                        box[kb] = w_bf_t
                    return emit_scores

                def mk_av(kb, qsA=qsA, qsB=qsB, lastA=lastA, lastB=lastB,
                          o_A=o_A, o_B=o_B, box=box):
                    def emit_av():
                        if kb <= lastA:
                            nc.tensor.matmul(
                                o_A, lhsT=vslice[:, kb, :],
                                rhs=box[kb][:, 0, :],
                                start=(kb == 0), stop=(kb == lastA))
                        nc.tensor.matmul(
                            o_B, lhsT=vslice[:, kb, :],
                            rhs=box[kb][:, 1, :],
                            start=(kb == 0), stop=(kb == lastB))
                        del box[kb]
                        if kb == lastA:
                            normalize_evict(b, h, qsA, o_A)
                        if kb == lastB:
                            normalize_evict(b, h, qsB, o_B)
                    return emit_av

                for kb in range(lastB + 1):
                    yield mk_scores(kb), mk_av(kb)

        def run_attention_pipeline(blocks, fillers=(), actions=None):
            """blocks: list of generators from attention_steps. Runs one
            DEPTH-deep pipeline across all of them. Fillers (extra PE work)
            are injected one-per-step and must all be emitted before the
            third block starts (its inputs come from the fillers). actions
            maps a global step index k -> thunks emitted right after step
            k's AV (used for collective triggers, gathers, out_proj passes).
            """
            steps = []
            bounds = []
            for blk in blocks:
                blksteps = list(blk)
                steps.extend(blksteps)
                bounds.append(len(steps))
            n = len(steps)
            acts = actions or {}
            fillers = list(fillers)
            fi = 0
            # fillers spread proportionally across the first two blocks
            # (their outputs feed block 3); even spreading keeps every block
            # short so the A2A triggers stay evenly spaced.
            span = max(1, bounds[1] - DEPTH - 2)
            for i in range(n + DEPTH):
                if i < n:
                    steps[i][0]()          # scores/exp/mask for step i
                want = min(len(fillers), ((i + 1) * len(fillers)) // span)
                while fi < want:
                    fillers[fi]()
                    fi += 1
                if i >= DEPTH:
                    k = i - DEPTH
                    steps[k][1]()          # AV for step k
                    for t in acts.get(k, ()):
                        t()
            while fi < len(fillers):
                fillers[fi]()
                fi += 1
            return bounds

        def exchange(h, b):
            """Trigger the (h, b) A2A (inputs already staged per-span)."""
            nc.gpsimd.collective_compute(
                "AllToAll", mybir.AluOpType.bypass,
                replica_groups=[list(range(8))],
                ins=[a2a_in[h, b].ap().opt()], outs=[a2a_out[h, b].ap().opt()],
            )

        def exchange11(half):
            """Trigger one 128KB half of the final (h1, b1) exchange."""
            nc.gpsimd.collective_compute(
                "AllToAll", mybir.AluOpType.bypass,
                replica_groups=[list(range(8))],
                ins=[a2a_in11[half].ap().opt()],
                outs=[a2a_out11[half].ap().opt()],
            )

        # ---- out_proj machinery -------------------------------------------
        # After the (h, b) A2A, slot c holds global head 2c+h's attn^T for my
        # 256 tokens of batch b. Slots are PAIRED into 128-partition tiles
        # (slot 2p in partitions 0:64, slot 2p+1 in 64:128), contracted
        # against wo_pack[h] whose rows are gathered the same way, and
        # accumulated over the 4 pairs -- a fully packed contraction.
        g_t = {}
        op_part = {}
        for h in range(H_PER):
            for b in range(B):
                g_t[h, b] = persist.tile([128, 4, TOK], BF16, tag=f"g{h}{b}",
                                         name=f"g{h}{b}")
        for b in range(B):
            op_part[b] = persist.tile([128, B * TOK // 128 // B, D],  # [128,2,D]
                                      BF16, tag=f"opart{b}",
                                      name=f"op_part{b}")
        wo_pack = [persist.tile([128, 4, D], BF16, tag=f"wop{h}",
                                name=f"wo_pack{h}") for h in range(H_PER)]
        bias_b = persist.tile([128, D], F32, tag="bias", name="bias_b")
        nc.scalar.dma_start(
            out=bias_b, in_=bo_t[0:1, :].to_broadcast([128, D]))

        def load_wo_pack(h, p):
            """wo_pack[h][t*64:(t+1)*64, p, :] = Wo rows of head 4p+h+2t."""
            for t in range(2):
                row = (4 * p + h + 2 * t) * DH
                st = wstage.tile([DH, D], F32, tag="wost", name="wost",
                                 bufs=2)
                nc.sync.dma_start(out=st, in_=wo_t[row:row + DH, :])
                nc.vector.tensor_copy(
                    out=wo_pack[h][t * DH:(t + 1) * DH, p, :], in_=st)

        def gather_g(h, b):
            # [8 slots, 64, TOK] -> [128, 4, TOK] with slot 2p+t at
            # partitions t*64:(t+1)*64, pair index p. Dispatched from the
            # gpsimd queue (waits on the A2A completion there, where nothing
            # critical queues behind it).
            nc.gpsimd.dma_start(
                out=g_t[h, b],
                in_=a2a_out[h, b].ap().rearrange("(pr t) p c -> (t p) pr c",
                                                 t=2))

        def gather_g11(half):
            nc.gpsimd.dma_start(
                out=g_t[1, 1][:, :, half * 128:(half + 1) * 128],
                in_=a2a_out11[half].ap().rearrange(
                    "(pr t) p c -> (t p) pr c", t=2))

        def mk_pass(h, b):
            """4 thunks; thunk (tb, dh) contracts the 4 packed pairs into
            psum for token block tb / out-column half dh. h==0 passes write
            bf16 partials (+bias); h==1 passes add the partials and DMA each
            finished 128-token block out as soon as both halves are done."""
            thunks = []
            ot = {}

            def mk(tb, dh):
                def run():
                    ps = ps_x.tile([128, 512], F32, tag="x", name="op_ps")
                    for p in range(4):
                        nc.tensor.matmul(
                            ps,
                            lhsT=g_t[h, b][:, p, tb * 128:(tb + 1) * 128],
                            rhs=wo_pack[h][:, p, dh * 512:(dh + 1) * 512],
                            start=(p == 0), stop=(p == 3))
                    if h == 0:
                        nc.vector.tensor_add(
                            op_part[b][:, tb, dh * 512:(dh + 1) * 512], ps,
                            bias_b[:, dh * 512:(dh + 1) * 512])
                    else:
                        if tb not in ot:
                            ot[tb] = opool.tile([128, D], F32, tag="ot",
                                                name=f"ot{b}{tb}")
                        nc.vector.tensor_add(
                            ot[tb][:, dh * 512:(dh + 1) * 512], ps,
                            op_part[b][:, tb, dh * 512:(dh + 1) * 512])
                        if dh == 1:
                            nc.scalar.dma_start(
                                out=out_t[b * TOK + tb * 128:
                                          b * TOK + (tb + 1) * 128, :],
                                in_=ot[tb])
                return run

            for tb in range(2):
                for dh in range(2):
                    thunks.append(mk(tb, dh))
            return thunks

        # ---- emission order ------------------------------------------------
        # weights first (small DMAs land in ~5us), then batch-0's transposes
        # INTERLEAVED with the projections that consume them. The PE queue is
        # strictly in-order, so emitting all 16 transpose blocks up front
        # head-of-line blocks the projection matmuls behind x DMAs that
        # haven't landed; interleaving keeps the PE dense from ~10us.
        xT0 = persist.tile([128, KT, S], BF16, tag="xT", name="xT0")
        qT0, kT0, vp0 = alloc_proj_tiles(0)
        ones_view0 = vp0.rearrange("p s (h c) -> p s h c",
                                   c=DH + 1)[:, :, :, DH:]
        nc.gpsimd.memset(ones_view0, 1.0)
        # first 4 x-blocks DMA first so the PE has transpose work from ~2us;
        # the weight stages land right behind them.
        for sb in range(4):
            transpose_x_step(0, xT0, sb)
        wq_bf, wk_bf, wv_bf = load_weights()
        for qb in range(4):
            for q in range(4):
                sb = qb * 4 + q
                if qb > 0:
                    transpose_x_step(0, xT0, sb)
                proj_v_step(xT0, wv_bf, vp0, sb)
            proj_qk_step(xT0, wq_bf, qT0, qb)
            proj_qk_step(xT0, wk_bf, kT0, qb)
            if qb == 0:
                # prewarm the ACT exp table set behind the first casts,
                # well before the first attention exp (~2.7us once)
                ewarm = const.tile([128, 16], F32)
                nc.vector.memset(ewarm, 0.0)
                ewarm_o = const.tile([128, 16], F32)
                nc.scalar.activation(ewarm_o, ewarm,
                                     mybir.ActivationFunctionType.Exp,
                                     scale=1.0)

        # batch 1's x-transpose + projections become PE filler inside the
        # first half of the attention pipeline (they keep PE at 100% duty
        # while ACT works through the exp chain); wo_pack loads ride along.
        xT1 = persist.tile([128, KT, S], BF16, tag="xT", name="xT1")
        qT1, kT1, vp1 = alloc_proj_tiles(1)
        ones_view1 = vp1.rearrange("p s (h c) -> p s h c",
                                   c=DH + 1)[:, :, :, DH:]
        nc.gpsimd.memset(ones_view1, 1.0)
        fillers = []
        for sb in range(SB):
            fillers.append(lambda sb=sb: transpose_x_step(1, xT1, sb))
        for w_bf, dest in ((wq_bf, qT1), (wk_bf, kT1)):
            for nt in range(NQS):
                fillers.append(
                    lambda w_bf=w_bf, dest=dest, nt=nt:
                        proj_qk_step(xT1, w_bf, dest, nt))
        for sb in range(SB):
            fillers.append(lambda sb=sb: proj_v_step(xT1, wv_bf, vp1, sb))
            if sb % 2 == 0:
                h, p = (sb // 2) % 2, sb // 4
                fillers.append(lambda h=h, p=p: load_wo_pack(h, p))

        # block order (h0,b0) (h1,b0) (h0,b1) (h1,b1): each block's A2A
        # triggers at its end (25/50/75/100%); its gather + out_proj pass
        # are scheduled one block later, when the exchange has landed.
        # steps per block: sum over q-span pairs qp of (lastB+1) = 8 + 16
        n_steps = sum(8 * qp + 8 for qp in range(NQS // 2))
        k0, k1, k2 = n_steps - 1, 2 * n_steps - 1, 3 * n_steps - 1
        pass00 = mk_pass(0, 0)
        pass10 = mk_pass(1, 0)
        # triggers are emitted BEFORE gathers on the gpsimd queue so a
        # gather's wait-for-collective never delays the next trigger; both
        # batch-0 gathers wait until the k2 boundary, after which the gpsimd
        # queue has no masking work left (block 4 masks on DVE), so their
        # collective waits can't stall the attention pipeline.
        pass01 = mk_pass(0, 1)
        pass11 = mk_pass(1, 1)
        actions = {
            k0: [lambda: exchange(0, 0)],
            k1: [lambda: exchange(1, 0)],
            k2: [lambda: exchange(0, 1), lambda: gather_g(0, 0),
                 lambda: gather_g(1, 0)],
            k2 + 2: [pass00[0]], k2 + 4: [pass00[1]],
            k2 + 6: [pass00[2]],
            # span 1 of (h1,b1) is staged by AV(k2+8): fire half A of the
            # final exchange immediately so it completes inside the pipeline.
            k2 + 8: [pass00[3], lambda: exchange11(0)],
            k2 + 10: [pass10[0]], k2 + 12: [pass10[1]],
            k2 + 14: [pass10[2]], k2 + 16: [pass10[3],
                                            lambda: gather_g(0, 1)],
            k2 + 20: [lambda: gather_g11(0)],
        }
        run_attention_pipeline(
            [
                attention_steps(0, 0, qT0, kT0, vp0),
                attention_steps(1, 0, qT0, kT0, vp0),
                attention_steps(0, 1, qT1, kT1, vp1),
                attention_steps(1, 1, qT1, kT1, vp1),
            ],
            fillers=fillers,
            actions=actions,
        )
        # tail: trigger half B of the final exchange; while it flies, the PE
        # runs the (h0,b1) pass and the half-A part of the (h1,b1) pass.
        # Only a 128KB exchange + one 4K-cycle pass trail everything.
        exchange11(1)
        for t in pass01:
            t()
        pass11[0]()
        pass11[1]()
        gather_g11(1)
        pass11[2]()
        pass11[3]()

    nc.compile()
    return nc


def shard_inputs(x, Wq, Wk, Wv, Wo, bo):
    """Full inputs -> per-core in_maps."""
    x = np.ascontiguousarray(np.asarray(x, dtype=np.float32))
    Wq = np.asarray(Wq, dtype=np.float32)
    Wk = np.asarray(Wk, dtype=np.float32)
    Wv = np.asarray(Wv, dtype=np.float32)
    Wo = np.ascontiguousarray(np.asarray(Wo, dtype=np.float32))
    bo = np.asarray(bo, dtype=np.float32).reshape(1, D)
    in_maps = []
    for c in range(N_CORES):
        cols = slice(c * DCOL, (c + 1) * DCOL)
        in_maps.append({
            "x": x,
            "wq": np.ascontiguousarray(Wq[:, cols]),
            "wk": np.ascontiguousarray(Wk[:, cols]),
            "wv": np.ascontiguousarray(Wv[:, cols]),
            "wo": Wo,
            "bo": bo,
        })
    return in_maps


def assemble_output(results):
    """Per-core out slices -> full [B, S, D]. Core c returns batch-0 tokens
    [256c, 256c+256) in rows 0:256; batch-1 tokens [128c, 128c+128) in rows
    256:384 and [1024+128c, 1024+128c+128) in rows 384:512 (the final
    exchange is split into two halves covering all 8 destinations each)."""
    out = np.empty((B, S, D), dtype=np.float32)
    for c in range(N_CORES):
        r = results[c]["out"]
        out[0, c * TOK:(c + 1) * TOK, :] = r[0:TOK]
        out[1, c * 128:(c + 1) * 128, :] = r[TOK:TOK + 128]
        out[1, 1024 + c * 128:1024 + (c + 1) * 128, :] = r[TOK + 128:2 * TOK]
    return out


def kernel(x, Wq, Wk, Wv, Wo, bo):
    if "nc" not in _CACHE:
        _CACHE["nc"] = build()
    nc = _CACHE["nc"]
    in_maps = shard_inputs(x, Wq, Wk, Wv, Wo, bo)
    res = run_bass_kernel_spmd(nc, in_maps, core_ids=list(range(N_CORES)))
    return assemble_output(res.results)


# revision 64
# speedup vs baseline: 1.1087x; 1.1087x over previous
"""Multi-head causal attention on 8 Trainium2 NeuronCores.

Problem: x [2, 2048, 1024] f32; Wq/Wk/Wv [1024, 1024]; Wo [1024, 1024]; bo [1024].
  q/k/v = split_heads(x @ W*)  (16 heads, head_dim 64)
  scores = q k^T, causal mask, / sqrt(1024), softmax, out = (w v) @ Wo + bo

Sharding: tensor-parallel over heads, 8-way (Megatron-style): core c computes
heads {2c, 2c+1} for BOTH batches. The concat+out_proj needs all heads, so
cores exchange attention outputs with 8-rank AllToAlls. Restructure vs the
296us baseline (whose tail after attention was ~75us: exposed A2As + a
zero-padded double out_proj at half clock) -- ~278us typical, best 263us:

 - per-(head,batch) AllToAlls triggered at the 25/50/75/100% marks of the
   attention pipeline (block order h0b0, h1b0, h0b1, h1b1). Triggered
   on-time and so 8-core-synchronized, each 256KB exchange completes in
   ~6us (vs ~30us when skewed). The LAST exchange (h1,b1) is further split
   into two 128KB halves (batch-1 tokens interleaved per-slot so each half
   covers all 8 destinations): half A + most out_proj work overlap the
   pipeline; only a ~6us exchange + one 4K-cycle pass trail it.
 - out_proj split into per-exchange partial passes, run as PE filler work
   inside the attention pipeline as each A2A lands.
 - out_proj contraction fully packed: the gathered 64-row head chunks are
   paired into 128-partition stationaries (chunk 2p in partitions 0:64,
   chunk 2p+1 in 64:128) against a row-gathered Wo ("wo_pack"), so out_proj
   costs 32768 PE cycles total instead of 65536 zero-padded ones.
 - output resharded: core j returns batch-0 tokens [256j,256j+256) and
   batch-1 tokens [128j,+128) and [1024+128j,+128); host reassembles.
 - A2A staging rides the sync queue (kept free of bulk loads: b1 x loads
   dispatch from the scalar queue), collective triggers ALWAYS precede
   gathers on the gpsimd queue, and the final block's causal masks run on
   DVE -- all so no collective-completion wait ever delays a trigger, a
   mask, or staging (each such coupling measured 30-70us).

Measured dead ends kept out: fp8 DoubleRow for proj/AV (this problem's
attention output is a near-uniform mean of ~random values, so the signal
averages down as fast as fp8 noise: 2-4% rel err vs the 2% budget); DMA
xbar transpose for x^T (serializes against collectives, destroying A2A
overlap); gpsimd casts (4x slower than DVE); --enable-ldw-opt (rejects
is_transpose/1-partition LDWEIGHTS).

On-chip layout trick (unchanged): attention is computed fully transposed
(scores^T = K Q^T in [k, q] layout) so the softmax weights come out exactly in
the layout the attn-value matmul wants as its moving operand, and the AV
result comes out as attn^T [d, q] which is exactly the stationary layout
out_proj wants. The softmax denominator is obtained for free by augmenting V
with a ones-column (row 64 of the AV psum accumulates sum(w)).

Compute dtype bf16 (fp32 accumulation in PSUM).
"""

from contextlib import ExitStack

import numpy as np

import concourse.bass as bass
import concourse.tile as tile
from concourse import bacc, mybir
from concourse.bass_utils import run_bass_kernel_spmd
from concourse.masks import make_identity

F32 = mybir.dt.float32
BF16 = mybir.dt.bfloat16

N_CORES = 8
B = 2
S = 2048
D = 1024
H = 16
DH = 64
H_PER = 2              # heads per core
DCOL = H_PER * DH      # 128: projection output cols per core
KT = D // 128          # 8 contraction tiles
SB = S // 128          # 16 sequence blocks
NQS = S // 512         # 4 q-spans
TOK = S // 8           # 256 output tokens per core per batch
SCALE = 1.0 / np.sqrt(np.float32(D))

_CACHE = {}


def build():
    """Build the SPMD program (identical on all 8 cores)."""
    nc = bacc.Bacc("TRN2", target_bir_lowering=False, debug=False)

    x_t = nc.dram_tensor("x", [B, S, D], F32, kind="ExternalInput")
    wq_t = nc.dram_tensor("wq", [D, DCOL], F32, kind="ExternalInput")
    wk_t = nc.dram_tensor("wk", [D, DCOL], F32, kind="ExternalInput")
    wv_t = nc.dram_tensor("wv", [D, DCOL], F32, kind="ExternalInput")
    wo_t = nc.dram_tensor("wo", [D, D], F32, kind="ExternalInput")
    bo_t = nc.dram_tensor("bo", [1, D], F32, kind="ExternalInput")
    # rows 0:TOK = batch-0 tokens [256c, 256c+256), rows TOK:2TOK = batch 1
    out_t = nc.dram_tensor("out", [B * TOK, D], F32, kind="ExternalOutput")

    # collective buffers (internal DRAM): one A2A per (local head, batch).
    # slot j of a2a_in goes to rank j = that head's attn^T restricted to
    # tokens [256j, 256j+256) of that batch; slot c of a2a_out arrived from
    # rank c = global head 2c+h for MY 256-token slice.
    warm_in = nc.dram_tensor("warm_in", [8, 16], F32)
    warm_out = nc.dram_tensor("warm_out", [8, 16], F32)
    a2a_in = {}
    a2a_out = {}
    for h in range(H_PER):
        for b in range(B):
            a2a_in[h, b] = nc.dram_tensor(f"a2a_in{h}{b}", [8, DH, TOK], BF16)
            a2a_out[h, b] = nc.dram_tensor(f"a2a_out{h}{b}", [8, DH, TOK], BF16)
    # the LAST exchange (h1, b1) is split into two 128KB halves so the first
    # half (+ its out_proj work) overlaps the pipeline tail; batch-1 tokens
    # are interleaved per-slot (slot j: tokens [128j,128j+128) of each
    # 1024-token half) so each half covers all 8 destinations.
    a2a_in11 = [nc.dram_tensor(f"a2a_in11{x}", [8, DH, 128], BF16)
                for x in "ab"]
    a2a_out11 = [nc.dram_tensor(f"a2a_out11{x}", [8, DH, 128], BF16)
                 for x in "ab"]

    with tile.TileContext(nc) as tc, ExitStack() as ctx:
        const = ctx.enter_context(tc.tile_pool(name="const", bufs=1))
        persist = ctx.enter_context(tc.tile_pool(name="persist", bufs=1))
        stage = ctx.enter_context(tc.tile_pool(name="stage", bufs=2))
        wstage = ctx.enter_context(tc.tile_pool(name="wstage", bufs=1))
        wpool = ctx.enter_context(tc.tile_pool(name="wpool", bufs=6))
        rbpool = ctx.enter_context(tc.tile_pool(name="rbpool", bufs=2))
        spool = ctx.enter_context(tc.tile_pool(name="spool", bufs=2))
        opool = ctx.enter_context(tc.tile_pool(name="opool", bufs=2))
        ps_mm = ctx.enter_context(tc.tile_pool(name="ps_mm", bufs=2, space="PSUM"))
        ps_o = ctx.enter_context(tc.tile_pool(name="ps_o", bufs=2, space="PSUM"))
        ps_x = ctx.enter_context(tc.tile_pool(name="ps_x", bufs=2, space="PSUM"))

        # ---- warmup collective: absorbs the per-execution ncfw entry cost
        # concurrently with the compute phase.
        nc.gpsimd.collective_compute(
            "AllToAll", mybir.AluOpType.bypass,
            replica_groups=[list(range(8))],
            ins=[warm_in.ap().opt()], outs=[warm_out.ap().opt()],
        )

        identity = const.tile([128, 128], BF16)
        make_identity(nc, identity)
        ones1 = const.tile([1, DH], BF16)
        nc.vector.memset(ones1, 1.0)
        # 4 causal masks (mask[r][p,f] = 1 iff f - p - 128r >= 0) for the
        # final attention block, whose masking runs on DVE instead of gpsimd
        # so the gpsimd queue (collective triggers + gathers) is never a
        # masking dependency at the tail.
        masks = []
        for r in range(4):
            mk = const.tile([128, 512], BF16, name=f"mask{r}")
            nc.vector.memset(mk, 1.0)
            nc.gpsimd.affine_select(
                out=mk, in_=mk,
                pattern=[[1, 512]],
                compare_op=mybir.AluOpType.is_ge,
                fill=0.0,
                base=-128 * r,
                channel_multiplier=-1)
            masks.append(mk)

        attnT = [
            persist.tile([128, S], BF16, tag=f"attnT{b}", name=f"attnT{b}")
            for b in range(B)
        ]

        # ---- phase helpers -------------------------------------------------
        def transpose_x_step(b, xT, sb):
            """One 128-row block of x[b] -> x^T bf16 columns, via PE
            transpose. Batch 0's cast runs on the otherwise-idle ACT."""
            xn = stage.tile([128, D], F32, tag="xn", name="xn", bufs=4)
            # b1's bulk loads dispatch from the scalar queue so the sync
            # queue (A2A staging) never backs up behind 8MB of x traffic;
            # their buffer-free waits resolve via DVE casts, which never
            # depend on later scalar-queue work (no cross-queue cycle).
            (nc.sync if b == 0 else nc.scalar).dma_start(
                out=xn, in_=x_t[b, sb * 128:(sb + 1) * 128, :])
            xb = stage.tile([128, D], BF16, tag=f"xb{b}", name="xb", bufs=2)
            # b0's cast runs on the otherwise-idle ACT; b1's on DVE (gpsimd
            # casts measured 4x slower, and ACT is exp-critical by then).
            if b == 0:
                nc.scalar.copy(xb, xn)
            else:
                nc.vector.tensor_copy(out=xb, in_=xn)
            for g in range(2):  # 2 groups of 4 d-blocks -> one psum bank
                pt = ps_x.tile([128, 4, 128], BF16, tag="x", name="pt")
                for k in range(4):
                    kt = g * 4 + k
                    nc.tensor.transpose(
                        pt[:, k, :], xb[:, kt * 128:(kt + 1) * 128], identity)
                dst = xT[:, g * 4:(g + 1) * 4, sb * 128:(sb + 1) * 128]
                nc.vector.tensor_copy(out=dst, in_=pt)

        def load_weights():
            # separate stage buffers per weight: with a single shared buffer
            # the three loads serialize DMA->cast->DMA->cast (~23us) and the
            # batch-0 x blocks queue behind them on the sync queue.
            def load_cast(dram_ap, kt_cols, name):
                st = wstage.tile([128, KT, kt_cols], F32, tag=f"wst_{name}",
                                 name="wst")
                nc.sync.dma_start(
                    out=st, in_=dram_ap.rearrange("(kt p) c -> p kt c", p=128))
                bf = persist.tile([128, KT, kt_cols], BF16, tag=name, name=name)
                nc.vector.tensor_copy(out=bf, in_=st)
                return bf

            wq_bf = load_cast(wq_t[:, :], DCOL, "wq")
            wk_bf = load_cast(wk_t[:, :], DCOL, "wk")
            wv_bf = load_cast(wv_t[:, :], DCOL, "wv")
            return wq_bf, wk_bf, wv_bf

        def alloc_proj_tiles(b):
            # Q^T / K^T are stored per-head, zero-padded to 128 partitions
            # (rows 64:128 = 0) because matmuls with contraction dim <= 64
            # run at HALF rate on the PE.
            qT = [persist.tile([128, S], BF16, tag=f"qT{b}{h}",
                               name=f"qT{b}{h}") for h in range(H_PER)]
            kTt = [persist.tile([128, S], BF16, tag=f"kT{b}{h}",
                                name=f"kT{b}{h}") for h in range(H_PER)]
            vp = persist.tile([128, SB, H_PER * (DH + 1)], BF16,
                              tag=f"vp{b}", name=f"vp{b}")
            for t in qT + kTt:
                nc.gpsimd.memset(t[DH:128, :], 0.0)
            return qT, kTt, vp

        def proj_qk_step(xT, w_bf, dest, nt):
            ps = ps_x.tile([128, 512], F32, tag="x", name="ps")
            for kt in range(KT):
                nc.tensor.matmul(
                    ps, lhsT=w_bf[:, kt, :],
                    rhs=xT[:, kt, nt * 512:(nt + 1) * 512],
                    start=(kt == 0), stop=(kt == KT - 1))
            for h in range(H_PER):
                nc.vector.tensor_copy(
                    out=dest[h][0:DH, nt * 512:(nt + 1) * 512],
                    in_=ps[h * DH:(h + 1) * DH, :])

        def proj_v_step(xT, wv_bf, vp, sb):
            ps = ps_x.tile([128, 512], F32, tag="x", name="ps")
            for kt in range(KT):
                nc.tensor.matmul(
                    ps[:, 0:DCOL], lhsT=xT[:, kt, sb * 128:(sb + 1) * 128],
                    rhs=wv_bf[:, kt, :],
                    start=(kt == 0), stop=(kt == KT - 1))
            dst = vp.rearrange("p s (h c) -> p s h c", c=DH + 1)[:, sb, :, :DH]
            nc.vector.tensor_copy(
                out=dst, in_=ps[:, 0:DCOL].rearrange("p (h c) -> p h c", c=DH))

        # Attention, software-pipelined GLOBALLY across (batch, head, q-span)
        # in PAIRS of k-blocks: scores for a pair land in a 2-bank psum tile,
        # ONE exp covers both halves, and the AV matmuls trail DEPTH pairs
        # behind, so the PE stream has a single priming point for the whole
        # attention phase.
        DEPTH = 4

        def normalize_evict(b, h, qs, o_ps):
            hr = h * DH
            denom = spool.tile([1, 512], F32, tag="den", name="denom")
            nc.vector.tensor_copy(out=denom, in_=o_ps[DH:DH + 1, :])
            recip_f = spool.tile([1, 512], F32, tag="recf", name="recip_f")
            nc.vector.reciprocal_approx_fast(out=recip_f, in_=denom)
            recip = spool.tile([1, 512], BF16, tag="rec", name="recip")
            nc.vector.tensor_copy(out=recip, in_=recip_f)
            # broadcast recip across the 64 partitions through the PE array
            # (the only cross-partition fabric that isn't a DMA round-trip).
            rb_ps = ps_x.tile([DH, 512], F32, tag="x", name="rb_ps")
            nc.tensor.matmul(rb_ps, lhsT=ones1, rhs=recip,
                             start=True, stop=True)
            rb = rbpool.tile([DH, 512], F32, tag="rb", name="rb")
            nc.vector.tensor_copy(out=rb, in_=rb_ps)
            dst = attnT[b][hr:hr + DH, qs * 512:(qs + 1) * 512]
            nc.vector.tensor_mul(dst, o_ps[0:DH, :], rb)
            # stage this span's attn^T to the A2A input right away. b0:
            # slot j = contiguous 256-token slice j (span covers slots 2qs,
            # 2qs+1). b1: interleaved map -- token t<1024 goes to slot
            # t//128 col t%128 (half A), t>=1024 to slot (t-1024)//128 col
            # 128+ (half B) -- so each half covers all 8 destinations.
            if b == 0:
                for j in range(2):
                    nc.sync.dma_start(
                        out=a2a_in[h, b][2 * qs + j],
                        in_=dst[:, j * TOK:(j + 1) * TOK])
            else:
                half, base = qs // 2, 4 * (qs % 2)
                for i in range(4):
                    if h == 0:
                        out_ap = a2a_in[0, 1][base + i, :,
                                              half * 128:(half + 1) * 128]
                    else:
                        out_ap = a2a_in11[half][base + i]
                    nc.sync.dma_start(
                        out=out_ap, in_=dst[:, i * 128:(i + 1) * 128])

        def attention_steps(h, b, qT, kTt, vp):
            """Yield (emit_scores, emit_av) closures, one pair per k-block,
            processing TWO q-spans at once (same stationary operand ->
            LDWEIGHTS dedups between the two matmuls). Epilogues fire from
            the AV closure that completes each span."""
            qTh, kTh = qT[h], kTt[h]
            vslice = vp[:, :, h * (DH + 1):(h + 1) * (DH + 1)]
            for qp in range(NQS // 2):
                qsA, qsB = 2 * qp, 2 * qp + 1
                lastA, lastB = 4 * qsA + 3, 4 * qsB + 3
                o_A = ps_o.tile([DH + 1, 512], F32, tag="o", name="o_A")
                o_B = ps_o.tile([DH + 1, 512], F32, tag="o", name="o_B")
                box = {}

                def mk_scores(kb, qsA=qsA, qsB=qsB, lastA=lastA, box=box):
                    def emit_scores():
                        s_ps = ps_mm.tile([128, 2, 512], F32, tag="mm",
                                          name="s_ps")
                        spans = ([(0, qsA)] if kb <= lastA else []) + [(1, qsB)]
                        for i, qs in spans:
                            nc.tensor.matmul(
                                s_ps[:, i, :],
                                lhsT=kTh[:, kb * 128:(kb + 1) * 128],
                                rhs=qTh[:, qs * 512:(qs + 1) * 512],
                                start=True, stop=True)
                        w_bf_t = wpool.tile([128, 2, 512], BF16, tag="w",
                                            name="w_bf_t")
                        if kb < 4 * qsA and len(spans) == 2:
                            # clean interior for both spans: one big exp
                            nc.scalar.activation(
                                w_bf_t, s_ps, mybir.ActivationFunctionType.Exp,
                                scale=float(SCALE))
                        else:
                            for i, qs in spans:
                                lo = max(0, 128 * (kb - 4 * qs))
                                nc.scalar.activation(
                                    w_bf_t[:, i, lo:512], s_ps[:, i, lo:512],
                                    mybir.ActivationFunctionType.Exp,
                                    scale=float(SCALE))
                        for i, qs in spans:
                            if kb >= 4 * qs:
                                # causal: keep iff (512qs+f) - (128kb+p) >= 0.
                                # Final block masks on DVE so the gpsimd
                                # queue (gathers + triggers) is never a
                                # masking dependency; the stale [0:lo] region
                                # is finite (prior exp outputs) so mul-by-0
                                # safely zeroes it.
                                if h == 1 and b == 1:
                                    nc.vector.tensor_mul(
                                        w_bf_t[:, i, :], w_bf_t[:, i, :],
                                        masks[kb - 4 * qs])
                                else:
                                    nc.gpsimd.affine_select(
                                        out=w_bf_t[:, i, :],
                                        in_=w_bf_t[:, i, :],
                                        pattern=[[1, 512]],
                                        compare_op=mybir.AluOpType.is_ge,
                                        fill=0.0,
                                        base=512 * qs - 128 * kb,
                                        channel_multiplier=-1)
                        box[kb] = w_bf_t
                    return emit_scores

                def mk_av(kb, qsA=qsA, qsB=qsB, lastA=lastA, lastB=lastB,
                          o_A=o_A, o_B=o_B, box=box):
                    def emit_av():
                        if kb <= lastA:
                            nc.tensor.matmul(
                                o_A, lhsT=vslice[:, kb, :],
                                rhs=box[kb][:, 0, :],
                                start=(kb == 0), stop=(kb == lastA))
                        nc.tensor.matmul(
                            o_B, lhsT=vslice[:, kb, :],
                            rhs=box[kb][:, 1, :],
                            start=(kb == 0), stop=(kb == lastB))
                        del box[kb]
                        if kb == lastA:
                            normalize_evict(b, h, qsA, o_A)
                        if kb == lastB:
                            normalize_evict(b, h, qsB, o_B)
                    return emit_av

                for kb in range(lastB + 1):
                    yield mk_scores(kb), mk_av(kb)

        def run_attention_pipeline(blocks, fillers=(), actions=None):
            """blocks: list of generators from attention_steps. Runs one
            DEPTH-deep pipeline across all of them. Fillers (extra PE work)
            are injected one-per-step and must all be emitted before the
            third block starts (its inputs come from the fillers). actions
            maps a global step index k -> thunks emitted right after step
            k's AV (used for collective triggers, gathers, out_proj passes).
            """
            steps = []
            bounds = []
            for blk in blocks:
                blksteps = list(blk)
                steps.extend(blksteps)
                bounds.append(len(steps))
            n = len(steps)
            acts = actions or {}
            fillers = list(fillers)
            fi = 0
            # fillers spread proportionally across the first two blocks
            # (their outputs feed block 3); even spreading keeps every block
            # short so the A2A triggers stay evenly spaced.
            span = max(1, bounds[1] - DEPTH - 2)
            for i in range(n + DEPTH):
                if i < n:
                    steps[i][0]()          # scores/exp/mask for step i
                want = min(len(fillers), ((i + 1) * len(fillers)) // span)
                while fi < want:
                    fillers[fi]()
                    fi += 1
                if i >= DEPTH:
                    k = i - DEPTH
                    steps[k][1]()          # AV for step k
                    for t in acts.get(k, ()):
                        t()
            while fi < len(fillers):
                fillers[fi]()
                fi += 1
            return bounds

        def exchange(h, b):
            """Trigger the (h, b) A2A (inputs already staged per-span)."""
            nc.gpsimd.collective_compute(
                "AllToAll", mybir.AluOpType.bypass,
                replica_groups=[list(range(8))],
                ins=[a2a_in[h, b].ap().opt()], outs=[a2a_out[h, b].ap().opt()],
            )

        def exchange11(half):
            """Trigger one 128KB half of the final (h1, b1) exchange."""
            nc.gpsimd.collective_compute(
                "AllToAll", mybir.AluOpType.bypass,
                replica_groups=[list(range(8))],
                ins=[a2a_in11[half].ap().opt()],
                outs=[a2a_out11[half].ap().opt()],
            )

        # ---- out_proj machinery -------------------------------------------
        # After the (h, b) A2A, slot c holds global head 2c+h's attn^T for my
        # 256 tokens of batch b. Slots are PAIRED into 128-partition tiles
        # (slot 2p in partitions 0:64, slot 2p+1 in 64:128), contracted
        # against wo_pack[h] whose rows are gathered the same way, and
        # accumulated over the 4 pairs -- a fully packed contraction.
        g_t = {}
        op_part = {}
        for h in range(H_PER):
            for b in range(B):
                g_t[h, b] = persist.tile([128, 4, TOK], BF16, tag=f"g{h}{b}",
                                         name=f"g{h}{b}")
        for b in range(B):
            op_part[b] = persist.tile([128, B * TOK // 128 // B, D],  # [128,2,D]
                                      BF16, tag=f"opart{b}",
                                      name=f"op_part{b}")
        wo_pack = [persist.tile([128, 4, D], BF16, tag=f"wop{h}",
                                name=f"wo_pack{h}") for h in range(H_PER)]
        bias_b = persist.tile([128, D], F32, tag="bias", name="bias_b")
        nc.scalar.dma_start(
            out=bias_b, in_=bo_t[0:1, :].to_broadcast([128, D]))

        def load_wo_pack(h, p):
            """wo_pack[h][t*64:(t+1)*64, p, :] = Wo rows of head 4p+h+2t."""
            for t in range(2):
                row = (4 * p + h + 2 * t) * DH
                st = wstage.tile([DH, D], F32, tag="wost", name="wost",
                                 bufs=2)
                nc.sync.dma_start(out=st, in_=wo_t[row:row + DH, :])
                nc.vector.tensor_copy(
                    out=wo_pack[h][t * DH:(t + 1) * DH, p, :], in_=st)

        def gather_g(h, b):
            # [8 slots, 64, TOK] -> [128, 4, TOK] with slot 2p+t at
            # partitions t*64:(t+1)*64, pair index p. Dispatched from the
            # gpsimd queue (waits on the A2A completion there, where nothing
            # critical queues behind it).
            nc.gpsimd.dma_start(
                out=g_t[h, b],
                in_=a2a_out[h, b].ap().rearrange("(pr t) p c -> (t p) pr c",
                                                 t=2))

        def gather_g11(half):
            nc.gpsimd.dma_start(
                out=g_t[1, 1][:, :, half * 128:(half + 1) * 128],
                in_=a2a_out11[half].ap().rearrange(
                    "(pr t) p c -> (t p) pr c", t=2))

        def mk_pass(h, b):
            """4 thunks; thunk (tb, dh) contracts the 4 packed pairs into
            psum for token block tb / out-column half dh. h==0 passes write
            bf16 partials (+bias); h==1 passes add the partials and DMA each
            finished 128-token block out as soon as both halves are done."""
            thunks = []
            ot = {}

            def mk(tb, dh):
                def run():
                    ps = ps_x.tile([128, 512], F32, tag="x", name="op_ps")
                    for p in range(4):
                        nc.tensor.matmul(
                            ps,
                            lhsT=g_t[h, b][:, p, tb * 128:(tb + 1) * 128],
                            rhs=wo_pack[h][:, p, dh * 512:(dh + 1) * 512],
                            start=(p == 0), stop=(p == 3))
                    if h == 0:
                        nc.vector.tensor_add(
                            op_part[b][:, tb, dh * 512:(dh + 1) * 512], ps,
                            bias_b[:, dh * 512:(dh + 1) * 512])
                    else:
                        if tb not in ot:
                            ot[tb] = opool.tile([128, D], F32, tag="ot",
                                                name=f"ot{b}{tb}")
                        nc.vector.tensor_add(
                            ot[tb][:, dh * 512:(dh + 1) * 512], ps,
                            op_part[b][:, tb, dh * 512:(dh + 1) * 512])
                        if dh == 1:
                            nc.scalar.dma_start(
                                out=out_t[b * TOK + tb * 128:
                                          b * TOK + (tb + 1) * 128, :],
                                in_=ot[tb])
                return run

            for tb in range(2):
                for dh in range(2):
                    thunks.append(mk(tb, dh))
            return thunks

        # ---- emission order ------------------------------------------------
        # weights first (small DMAs land in ~5us), then batch-0's transposes
        # INTERLEAVED with the projections that consume them. The PE queue is
        # strictly in-order, so emitting all 16 transpose blocks up front
        # head-of-line blocks the projection matmuls behind x DMAs that
        # haven't landed; interleaving keeps the PE dense from ~10us.
        xT0 = persist.tile([128, KT, S], BF16, tag="xT", name="xT0")
        qT0, kT0, vp0 = alloc_proj_tiles(0)
        ones_view0 = vp0.rearrange("p s (h c) -> p s h c",
                                   c=DH + 1)[:, :, :, DH:]
        nc.gpsimd.memset(ones_view0, 1.0)
        # first 4 x-blocks DMA first so the PE has transpose work from ~2us;
        # the weight stages land right behind them.
        for sb in range(4):
            transpose_x_step(0, xT0, sb)
        wq_bf, wk_bf, wv_bf = load_weights()
        for qb in range(4):
            for q in range(4):
                sb = qb * 4 + q
                if qb > 0:
                    transpose_x_step(0, xT0, sb)
                proj_v_step(xT0, wv_bf, vp0, sb)
            proj_qk_step(xT0, wq_bf, qT0, qb)
            proj_qk_step(xT0, wk_bf, kT0, qb)
            if qb == 0:
                # prewarm the ACT exp table set behind the first casts,
                # well before the first attention exp (~2.7us once)
                ewarm = const.tile([128, 16], F32)
                nc.vector.memset(ewarm, 0.0)
                ewarm_o = const.tile([128, 16], F32)
                nc.scalar.activation(ewarm_o, ewarm,
                                     mybir.ActivationFunctionType.Exp,
                                     scale=1.0)

        # batch 1's x-transpose + projections become PE filler inside the
        # first half of the attention pipeline (they keep PE at 100% duty
        # while ACT works through the exp chain); wo_pack loads ride along.
        xT1 = persist.tile([128, KT, S], BF16, tag="xT", name="xT1")
        qT1, kT1, vp1 = alloc_proj_tiles(1)
        ones_view1 = vp1.rearrange("p s (h c) -> p s h c",
                                   c=DH + 1)[:, :, :, DH:]
        nc.gpsimd.memset(ones_view1, 1.0)
        fillers = []
        for sb in range(SB):
            fillers.append(lambda sb=sb: transpose_x_step(1, xT1, sb))
        for w_bf, dest in ((wq_bf, qT1), (wk_bf, kT1)):
            for nt in range(NQS):
                fillers.append(
                    lambda w_bf=w_bf, dest=dest, nt=nt:
                        proj_qk_step(xT1, w_bf, dest, nt))
        for sb in range(SB):
            fillers.append(lambda sb=sb: proj_v_step(xT1, wv_bf, vp1, sb))
            if sb % 2 == 0:
                h, p = (sb // 2) % 2, sb // 4
                fillers.append(lambda h=h, p=p: load_wo_pack(h, p))

        # block order (h0,b0) (h1,b0) (h0,b1) (h1,b1): each block's A2A
        # triggers at its end (25/50/75/100%); its gather + out_proj pass
        # are scheduled one block later, when the exchange has landed.
        # steps per block: sum over q-span pairs qp of (lastB+1) = 8 + 16
        n_steps = sum(8 * qp + 8 for qp in range(NQS // 2))
        k0, k1, k2 = n_steps - 1, 2 * n_steps - 1, 3 * n_steps - 1
        pass00 = mk_pass(0, 0)
        pass10 = mk_pass(1, 0)
        # triggers are emitted BEFORE gathers on the gpsimd queue so a
        # gather's wait-for-collective never delays the next trigger; both
        # batch-0 gathers wait until the k2 boundary, after which the gpsimd
        # queue has no masking work left (block 4 masks on DVE), so their
        # collective waits can't stall the attention pipeline.
        pass01 = mk_pass(0, 1)
        pass11 = mk_pass(1, 1)
        actions = {
            k0: [lambda: exchange(0, 0)],
            k1: [lambda: exchange(1, 0)],
            k2: [lambda: exchange(0, 1), lambda: gather_g(0, 0),
                 lambda: gather_g(1, 0)],
            k2 + 2: [pass00[0]], k2 + 4: [pass00[1]],
            k2 + 6: [pass00[2]],
            # span 1 of (h1,b1) is staged by AV(k2+8): fire half A of the
            # final exchange immediately so it completes inside the pipeline.
            k2 + 8: [pass00[3], lambda: exchange11(0)],
            k2 + 10: [pass10[0]], k2 + 12: [pass10[1]],
            k2 + 14: [pass10[2]], k2 + 16: [pass10[3],
                                            lambda: gather_g(0, 1)],
            k2 + 20: [lambda: gather_g11(0)],
        }
        run_attention_pipeline(
            [
                attention_steps(0, 0, qT0, kT0, vp0),
                attention_steps(1, 0, qT0, kT0, vp0),
                attention_steps(0, 1, qT1, kT1, vp1),
                attention_steps(1, 1, qT1, kT1, vp1),
            ],
            fillers=fillers,
            actions=actions,
        )
        # tail: trigger half B of the final exchange; while it flies, the PE
        # runs the (h0,b1) pass and the half-A part of the (h1,b1) pass.
        # Only a 128KB exchange + one 4K-cycle pass trail everything.
        exchange11(1)
        for t in pass01:
            t()
        pass11[0]()
        pass11[1]()
        gather_g11(1)
        pass11[2]()
        pass11[3]()

    nc.compile()
    return nc


def shard_inputs(x, Wq, Wk, Wv, Wo, bo):
    """Full inputs -> per-core in_maps."""
    x = np.ascontiguousarray(np.asarray(x, dtype=np.float32))
    Wq = np.asarray(Wq, dtype=np.float32)
    Wk = np.asarray(Wk, dtype=np.float32)
    Wv = np.asarray(Wv, dtype=np.float32)
    Wo = np.ascontiguousarray(np.asarray(Wo, dtype=np.float32))
    bo = np.asarray(bo, dtype=np.float32).reshape(1, D)
    in_maps = []
    for c in range(N_CORES):
        cols = slice(c * DCOL, (c + 1) * DCOL)
        in_maps.append({
            "x": x,
            "wq": np.ascontiguousarray(Wq[:, cols]),
            "wk": np.ascontiguousarray(Wk[:, cols]),
            "wv": np.ascontiguousarray(Wv[:, cols]),
            "wo": Wo,
            "bo": bo,
        })
    return in_maps


def assemble_output(results):
    """Per-core out slices -> full [B, S, D]. Core c returns batch-0 tokens
    [256c, 256c+256) in rows 0:256; batch-1 tokens [128c, 128c+128) in rows
    256:384 and [1024+128c, 1024+128c+128) in rows 384:512 (the final
    exchange is split into two halves covering all 8 destinations each)."""
    out = np.empty((B, S, D), dtype=np.float32)
    for c in range(N_CORES):
        r = results[c]["out"]
        out[0, c * TOK:(c + 1) * TOK, :] = r[0:TOK]
        out[1, c * 128:(c + 1) * 128, :] = r[TOK:TOK + 128]
        out[1, 1024 + c * 128:1024 + (c + 1) * 128, :] = r[TOK + 128:2 * TOK]
    return out


def kernel(x, Wq, Wk, Wv, Wo, bo):
    if "nc" not in _CACHE:
        _CACHE["nc"] = build()
    nc = _CACHE["nc"]
    in_maps = shard_inputs(x, Wq, Wk, Wv, Wo, bo)
    res = run_bass_kernel_spmd(nc, in_maps, core_ids=list(range(N_CORES)))
    return assemble_output(res.results)


# revision 65
# speedup vs baseline: 1.2290x; 1.1085x over previous
"""Multi-head causal attention on 8 Trainium2 NeuronCores.

Problem: x [2, 2048, 1024] f32; Wq/Wk/Wv [1024, 1024]; Wo [1024, 1024]; bo [1024].
  q/k/v = split_heads(x @ W*)  (16 heads, head_dim 64)
  scores = q k^T, causal mask, / sqrt(1024), softmax, out = (w v) @ Wo + bo

Sharding: tensor-parallel over heads, 8-way (Megatron-style): core c computes
heads {2c, 2c+1} for BOTH batches. The concat+out_proj needs all heads, so
cores exchange attention outputs with 8-rank AllToAlls. Restructure vs the
296us baseline (whose tail after attention was ~75us: exposed A2As + a
zero-padded double out_proj at half clock) -- ~278us typical, best 263us:

 - per-(head,batch) AllToAlls triggered at the 25/50/75/100% marks of the
   attention pipeline (block order h0b0, h1b0, h0b1, h1b1). Triggered
   on-time and so 8-core-synchronized, each 256KB exchange completes in
   ~6us (vs ~30us when skewed). The LAST exchange (h1,b1) is further split
   into two 128KB halves (batch-1 tokens interleaved per-slot so each half
   covers all 8 destinations): half A + most out_proj work overlap the
   pipeline; only a ~6us exchange + one 4K-cycle pass trail it.
 - out_proj split into per-exchange partial passes, run as PE filler work
   inside the attention pipeline as each A2A lands.
 - out_proj contraction fully packed: the gathered 64-row head chunks are
   paired into 128-partition stationaries (chunk 2p in partitions 0:64,
   chunk 2p+1 in 64:128) against a row-gathered Wo ("wo_pack"), so out_proj
   costs 32768 PE cycles total instead of 65536 zero-padded ones.
 - output resharded: core j returns batch-0 tokens [256j,256j+256) and
   batch-1 tokens [128j,+128) and [1024+128j,+128); host reassembles.
 - A2A staging rides the sync queue (kept free of bulk loads: b1 x loads
   dispatch from the scalar queue), collective triggers ALWAYS precede
   gathers on the gpsimd queue, and the final block's causal masks run on
   DVE -- all so no collective-completion wait ever delays a trigger, a
   mask, or staging (each such coupling measured 30-70us).

Measured dead ends kept out: fp8 DoubleRow for proj/AV (this problem's
attention output is a near-uniform mean of ~random values, so the signal
averages down as fast as fp8 noise: 2-4% rel err vs the 2% budget); DMA
xbar transpose for x^T (serializes against collectives, destroying A2A
overlap); gpsimd casts (4x slower than DVE); --enable-ldw-opt (rejects
is_transpose/1-partition LDWEIGHTS).

On-chip layout trick (unchanged): attention is computed fully transposed
(scores^T = K Q^T in [k, q] layout) so the softmax weights come out exactly in
the layout the attn-value matmul wants as its moving operand, and the AV
result comes out as attn^T [d, q] which is exactly the stationary layout
out_proj wants. The softmax denominator is obtained for free by augmenting V
with a ones-column (row 64 of the AV psum accumulates sum(w)).

Compute dtype bf16 (fp32 accumulation in PSUM).
"""

from contextlib import ExitStack

import numpy as np

import concourse.bass as bass
import concourse.tile as tile
from concourse import bacc, mybir
from concourse.bass_utils import run_bass_kernel_spmd
from concourse.masks import make_identity

F32 = mybir.dt.float32
BF16 = mybir.dt.bfloat16

N_CORES = 8
B = 2
S = 2048
D = 1024
H = 16
DH = 64
H_PER = 2              # heads per core
DCOL = H_PER * DH      # 128: projection output cols per core
KT = D // 128          # 8 contraction tiles
SB = S // 128          # 16 sequence blocks
NQS = S // 512         # 4 q-spans
TOK = S // 8           # 256 output tokens per core per batch
SCALE = 1.0 / np.sqrt(np.float32(D))

_CACHE = {}


def build():
    """Build the SPMD program (identical on all 8 cores)."""
    nc = bacc.Bacc("TRN2", target_bir_lowering=False, debug=False)

    x_t = nc.dram_tensor("x", [B, S, D], F32, kind="ExternalInput")
    wq_t = nc.dram_tensor("wq", [D, DCOL], F32, kind="ExternalInput")
    wk_t = nc.dram_tensor("wk", [D, DCOL], F32, kind="ExternalInput")
    wv_t = nc.dram_tensor("wv", [D, DCOL], F32, kind="ExternalInput")
    wo_t = nc.dram_tensor("wo", [D, D], F32, kind="ExternalInput")
    bo_t = nc.dram_tensor("bo", [1, D], F32, kind="ExternalInput")
    # rows 0:TOK = batch-0 tokens [256c, 256c+256), rows TOK:2TOK = batch 1
    out_t = nc.dram_tensor("out", [B * TOK, D], F32, kind="ExternalOutput")

    # collective buffers (internal DRAM): one A2A per (local head, batch).
    # slot j of a2a_in goes to rank j = that head's attn^T restricted to
    # tokens [256j, 256j+256) of that batch; slot c of a2a_out arrived from
    # rank c = global head 2c+h for MY 256-token slice.
    warm_in = nc.dram_tensor("warm_in", [8, 16], F32)
    warm_out = nc.dram_tensor("warm_out", [8, 16], F32)
    a2a_in = {}
    a2a_out = {}
    for h in range(H_PER):
        for b in range(B):
            a2a_in[h, b] = nc.dram_tensor(f"a2a_in{h}{b}", [8, DH, TOK], BF16)
            a2a_out[h, b] = nc.dram_tensor(f"a2a_out{h}{b}", [8, DH, TOK], BF16)
    # the LAST exchange (h1, b1) is split into two 128KB halves so the first
    # half (+ its out_proj work) overlaps the pipeline tail; batch-1 tokens
    # are interleaved per-slot (slot j: tokens [128j,128j+128) of each
    # 1024-token half) so each half covers all 8 destinations.
    a2a_in11 = [nc.dram_tensor(f"a2a_in11{x}", [8, DH, 128], BF16)
                for x in "ab"]
    a2a_out11 = [nc.dram_tensor(f"a2a_out11{x}", [8, DH, 128], BF16)
                 for x in "ab"]
    rb_scr = [nc.dram_tensor(f"rbscr{i}", [1, 512], BF16) for i in range(2)]

    with tile.TileContext(nc) as tc, ExitStack() as ctx:
        const = ctx.enter_context(tc.tile_pool(name="const", bufs=1))
        persist = ctx.enter_context(tc.tile_pool(name="persist", bufs=1))
        stage = ctx.enter_context(tc.tile_pool(name="stage", bufs=2))
        wstage = ctx.enter_context(tc.tile_pool(name="wstage", bufs=1))
        wpool = ctx.enter_context(tc.tile_pool(name="wpool", bufs=6))
        rbpool = ctx.enter_context(tc.tile_pool(name="rbpool", bufs=2))
        spool = ctx.enter_context(tc.tile_pool(name="spool", bufs=2))
        opool = ctx.enter_context(tc.tile_pool(name="opool", bufs=2))
        ps_mm = ctx.enter_context(tc.tile_pool(name="ps_mm", bufs=2, space="PSUM"))
        ps_o = ctx.enter_context(tc.tile_pool(name="ps_o", bufs=2, space="PSUM"))
        ps_x = ctx.enter_context(tc.tile_pool(name="ps_x", bufs=2, space="PSUM"))

        # ---- warmup collective: absorbs the per-execution ncfw entry cost
        # concurrently with the compute phase.
        nc.gpsimd.collective_compute(
            "AllToAll", mybir.AluOpType.bypass,
            replica_groups=[list(range(8))],
            ins=[warm_in.ap().opt()], outs=[warm_out.ap().opt()],
        )

        identity = const.tile([128, 128], BF16)
        make_identity(nc, identity)
        ones1 = const.tile([1, DH], BF16)
        nc.vector.memset(ones1, 1.0)
        # 4 causal masks (mask[r][p,f] = 1 iff f - p - 128r >= 0) for the
        # final attention block, whose masking runs on DVE instead of gpsimd
        # so the gpsimd queue (collective triggers + gathers) is never a
        # masking dependency at the tail.
        masks = []
        for r in range(4):
            mk = const.tile([128, 512], BF16, name=f"mask{r}")
            nc.vector.memset(mk, 1.0)
            nc.gpsimd.affine_select(
                out=mk, in_=mk,
                pattern=[[1, 512]],
                compare_op=mybir.AluOpType.is_ge,
                fill=0.0,
                base=-128 * r,
                channel_multiplier=-1)
            masks.append(mk)

        attnT = [
            persist.tile([128, S], BF16, tag=f"attnT{b}", name=f"attnT{b}")
            for b in range(B)
        ]

        rb_i = [0]
        # ---- phase helpers -------------------------------------------------
        def transpose_x_step(b, xT, sb):
            """One 128-row block of x[b] -> x^T bf16 columns, via PE
            transpose. Batch 0's cast runs on the otherwise-idle ACT."""
            xn = stage.tile([128, D], F32, tag="xn", name="xn", bufs=4)
            # b1's bulk loads dispatch from the scalar queue so the sync
            # queue (A2A staging) never backs up behind 8MB of x traffic;
            # their buffer-free waits resolve via DVE casts, which never
            # depend on later scalar-queue work (no cross-queue cycle).
            (nc.sync if b == 0 else nc.scalar).dma_start(
                out=xn, in_=x_t[b, sb * 128:(sb + 1) * 128, :])
            xb = stage.tile([128, D], BF16, tag=f"xb{b}", name="xb", bufs=2)
            # b0's cast runs on the otherwise-idle ACT; b1's on DVE (gpsimd
            # casts measured 4x slower, and ACT is exp-critical by then).
            if b == 0:
                nc.scalar.copy(xb, xn)
            else:
                nc.vector.tensor_copy(out=xb, in_=xn)
            for g in range(2):  # 2 groups of 4 d-blocks -> one psum bank
                pt = ps_x.tile([128, 4, 128], BF16, tag="x", name="pt")
                for k in range(4):
                    kt = g * 4 + k
                    nc.tensor.transpose(
                        pt[:, k, :], xb[:, kt * 128:(kt + 1) * 128], identity)
                dst = xT[:, g * 4:(g + 1) * 4, sb * 128:(sb + 1) * 128]
                nc.vector.tensor_copy(out=dst, in_=pt)

        def load_weights():
            def load_cast(dram_ap, kt_cols, name):
                st = wstage.tile([128, KT, kt_cols], F32, tag="wst",
                                 name="wst")
                nc.sync.dma_start(
                    out=st, in_=dram_ap.rearrange("(kt p) c -> p kt c", p=128))
                bf = persist.tile([128, KT, kt_cols], BF16, tag=name, name=name)
                nc.vector.tensor_copy(out=bf, in_=st)
                return bf

            wq_bf = load_cast(wq_t[:, :], DCOL, "wq")
            wk_bf = load_cast(wk_t[:, :], DCOL, "wk")
            wv_bf = load_cast(wv_t[:, :], DCOL, "wv")
            return wq_bf, wk_bf, wv_bf

        def alloc_proj_tiles(b):
            # Q^T / K^T are stored per-head, zero-padded to 128 partitions
            # (rows 64:128 = 0) because matmuls with contraction dim <= 64
            # run at HALF rate on the PE.
            qT = [persist.tile([128, S], BF16, tag=f"qT{b}{h}",
                               name=f"qT{b}{h}") for h in range(H_PER)]
            kTt = [persist.tile([128, S], BF16, tag=f"kT{b}{h}",
                                name=f"kT{b}{h}") for h in range(H_PER)]
            vp = persist.tile([128, SB, H_PER * (DH + 1)], BF16,
                              tag=f"vp{b}", name=f"vp{b}")
            for t in qT + kTt:
                nc.gpsimd.memset(t[DH:128, :], 0.0)
            return qT, kTt, vp

        def proj_qk_step(xT, w_bf, dest, nt):
            ps = ps_x.tile([128, 512], F32, tag="x", name="ps")
            for kt in range(KT):
                nc.tensor.matmul(
                    ps, lhsT=w_bf[:, kt, :],
                    rhs=xT[:, kt, nt * 512:(nt + 1) * 512],
                    start=(kt == 0), stop=(kt == KT - 1))
            for h in range(H_PER):
                nc.vector.tensor_copy(
                    out=dest[h][0:DH, nt * 512:(nt + 1) * 512],
                    in_=ps[h * DH:(h + 1) * DH, :])

        def proj_v_step(xT, wv_bf, vp, sb):
            ps = ps_x.tile([128, 512], F32, tag="x", name="ps")
            for kt in range(KT):
                nc.tensor.matmul(
                    ps[:, 0:DCOL], lhsT=xT[:, kt, sb * 128:(sb + 1) * 128],
                    rhs=wv_bf[:, kt, :],
                    start=(kt == 0), stop=(kt == KT - 1))
            dst = vp.rearrange("p s (h c) -> p s h c", c=DH + 1)[:, sb, :, :DH]
            nc.vector.tensor_copy(
                out=dst, in_=ps[:, 0:DCOL].rearrange("p (h c) -> p h c", c=DH))

        # Attention, software-pipelined GLOBALLY across (batch, head, q-span)
        # in PAIRS of k-blocks: scores for a pair land in a 2-bank psum tile,
        # ONE exp covers both halves, and the AV matmuls trail DEPTH pairs
        # behind, so the PE stream has a single priming point for the whole
        # attention phase.
        DEPTH = 4

        def normalize_evict(b, h, qs, o_ps):
            hr = h * DH
            denom = spool.tile([1, 512], F32, tag="den", name="denom")
            nc.vector.tensor_copy(out=denom, in_=o_ps[DH:DH + 1, :])
            recip_f = spool.tile([1, 512], F32, tag="recf", name="recip_f")
            nc.vector.reciprocal_approx_fast(out=recip_f, in_=denom)
            recip = spool.tile([1, 512], BF16, tag="rec", name="recip")
            nc.vector.tensor_copy(out=recip, in_=recip_f)
            # broadcast recip across the 64 partitions via a DRAM
            # round-trip broadcast-read (saves 16 PE matmuls + LDWEIGHTS +
            # 16 DVE psum evictions vs the ones-column PE broadcast; both
            # DMAs precede this span's staging in the sync queue, so the
            # collective trigger ordering is unchanged).
            scr = rb_scr[rb_i[0] % 2]
            rb_i[0] += 1
            nc.sync.dma_start(out=scr[0:1, :], in_=recip)
            rb = rbpool.tile([DH, 512], BF16, tag="rb", name="rb")
            nc.sync.dma_start(out=rb, in_=scr[0:1, :].to_broadcast([DH, 512]))
            dst = attnT[b][hr:hr + DH, qs * 512:(qs + 1) * 512]
            nc.vector.tensor_mul(dst, o_ps[0:DH, :], rb)
            # stage this span's attn^T to the A2A input right away. b0:
            # slot j = contiguous 256-token slice j (span covers slots 2qs,
            # 2qs+1). b1: interleaved map -- token t<1024 goes to slot
            # t//128 col t%128 (half A), t>=1024 to slot (t-1024)//128 col
            # 128+ (half B) -- so each half covers all 8 destinations.
            if b == 0:
                for j in range(2):
                    nc.sync.dma_start(
                        out=a2a_in[h, b][2 * qs + j],
                        in_=dst[:, j * TOK:(j + 1) * TOK])
            else:
                half, base = qs // 2, 4 * (qs % 2)
                for i in range(4):
                    if h == 0:
                        out_ap = a2a_in[0, 1][base + i, :,
                                              half * 128:(half + 1) * 128]
                    else:
                        out_ap = a2a_in11[half][base + i]
                    nc.sync.dma_start(
                        out=out_ap, in_=dst[:, i * 128:(i + 1) * 128])

        def attention_steps(h, b, qT, kTt, vp):
            """Yield (emit_scores, emit_av) closures, one pair per k-block,
            processing TWO q-spans at once (same stationary operand ->
            LDWEIGHTS dedups between the two matmuls). Epilogues fire from
            the AV closure that completes each span."""
            qTh, kTh = qT[h], kTt[h]
            vslice = vp[:, :, h * (DH + 1):(h + 1) * (DH + 1)]
            for qp in range(NQS // 2):
                qsA, qsB = 2 * qp, 2 * qp + 1
                lastA, lastB = 4 * qsA + 3, 4 * qsB + 3
                o_A = ps_o.tile([DH + 1, 512], F32, tag="o", name="o_A")
                o_B = ps_o.tile([DH + 1, 512], F32, tag="o", name="o_B")
                box = {}

                def mk_scores(kb, qsA=qsA, qsB=qsB, lastA=lastA, box=box):
                    def emit_scores():
                        s_ps = ps_mm.tile([128, 2, 512], F32, tag="mm",
                                          name="s_ps")
                        spans = ([(0, qsA)] if kb <= lastA else []) + [(1, qsB)]
                        for i, qs in spans:
                            nc.tensor.matmul(
                                s_ps[:, i, :],
                                lhsT=kTh[:, kb * 128:(kb + 1) * 128],
                                rhs=qTh[:, qs * 512:(qs + 1) * 512],
                                start=True, stop=True)
                        w_bf_t = wpool.tile([128, 2, 512], BF16, tag="w",
                                            name="w_bf_t")
                        if kb < 4 * qsA and len(spans) == 2:
                            # clean interior for both spans: one big exp
                            nc.scalar.activation(
                                w_bf_t, s_ps, mybir.ActivationFunctionType.Exp,
                                scale=float(SCALE))
                        else:
                            for i, qs in spans:
                                lo = max(0, 128 * (kb - 4 * qs))
                                nc.scalar.activation(
                                    w_bf_t[:, i, lo:512], s_ps[:, i, lo:512],
                                    mybir.ActivationFunctionType.Exp,
                                    scale=float(SCALE))
                        for i, qs in spans:
                            if kb >= 4 * qs:
                                # causal: keep iff (512qs+f) - (128kb+p) >= 0.
                                # Final block masks on DVE so the gpsimd
                                # queue (gathers + triggers) is never a
                                # masking dependency; the stale [0:lo] region
                                # is finite (prior exp outputs) so mul-by-0
                                # safely zeroes it.
                                if h == 1 and b == 1:
                                    nc.vector.tensor_mul(
                                        w_bf_t[:, i, :], w_bf_t[:, i, :],
                                        masks[kb - 4 * qs])
                                else:
                                    nc.gpsimd.affine_select(
                                        out=w_bf_t[:, i, :],
                                        in_=w_bf_t[:, i, :],
                                        pattern=[[1, 512]],
                                        compare_op=mybir.AluOpType.is_ge,
                                        fill=0.0,
                                        base=512 * qs - 128 * kb,
                                        channel_multiplier=-1)
                        box[kb] = w_bf_t
                    return emit_scores

                def mk_av(kb, qsA=qsA, qsB=qsB, lastA=lastA, lastB=lastB,
                          o_A=o_A, o_B=o_B, box=box):
                    def emit_av():
                        if kb <= lastA:
                            nc.tensor.matmul(
                                o_A, lhsT=vslice[:, kb, :],
                                rhs=box[kb][:, 0, :],
                                start=(kb == 0), stop=(kb == lastA))
                        nc.tensor.matmul(
                            o_B, lhsT=vslice[:, kb, :],
                            rhs=box[kb][:, 1, :],
                            start=(kb == 0), stop=(kb == lastB))
                        del box[kb]
                        if kb == lastA:
                            normalize_evict(b, h, qsA, o_A)
                        if kb == lastB:
                            normalize_evict(b, h, qsB, o_B)
                    return emit_av

                for kb in range(lastB + 1):
                    yield mk_scores(kb), mk_av(kb)

        def run_attention_pipeline(blocks, fillers=(), actions=None):
            """blocks: list of generators from attention_steps. Runs one
            DEPTH-deep pipeline across all of them. Fillers (extra PE work)
            are injected one-per-step and must all be emitted before the
            third block starts (its inputs come from the fillers). actions
            maps a global step index k -> thunks emitted right after step
            k's AV (used for collective triggers, gathers, out_proj passes).
            """
            steps = []
            bounds = []
            for blk in blocks:
                blksteps = list(blk)
                steps.extend(blksteps)
                bounds.append(len(steps))
            n = len(steps)
            acts = actions or {}
            fillers = list(fillers)
            fi = 0
            # fillers spread proportionally across the first two blocks
            # (their outputs feed block 3); even spreading keeps every block
            # short so the A2A triggers stay evenly spaced.
            span = max(1, bounds[1] - DEPTH - 2)
            for i in range(n + DEPTH):
                if i < n:
                    steps[i][0]()          # scores/exp/mask for step i
                want = min(len(fillers), ((i + 1) * len(fillers)) // span)
                while fi < want:
                    fillers[fi]()
                    fi += 1
                if i >= DEPTH:
                    k = i - DEPTH
                    steps[k][1]()          # AV for step k
                    for t in acts.get(k, ()):
                        t()
            while fi < len(fillers):
                fillers[fi]()
                fi += 1
            return bounds

        def exchange(h, b):
            """Trigger the (h, b) A2A (inputs already staged per-span)."""
            nc.gpsimd.collective_compute(
                "AllToAll", mybir.AluOpType.bypass,
                replica_groups=[list(range(8))],
                ins=[a2a_in[h, b].ap().opt()], outs=[a2a_out[h, b].ap().opt()],
            )

        def exchange11(half):
            """Trigger one 128KB half of the final (h1, b1) exchange."""
            nc.gpsimd.collective_compute(
                "AllToAll", mybir.AluOpType.bypass,
                replica_groups=[list(range(8))],
                ins=[a2a_in11[half].ap().opt()],
                outs=[a2a_out11[half].ap().opt()],
            )

        # ---- out_proj machinery -------------------------------------------
        # After the (h, b) A2A, slot c holds global head 2c+h's attn^T for my
        # 256 tokens of batch b. Slots are PAIRED into 128-partition tiles
        # (slot 2p in partitions 0:64, slot 2p+1 in 64:128), contracted
        # against wo_pack[h] whose rows are gathered the same way, and
        # accumulated over the 4 pairs -- a fully packed contraction.
        g_t = {}
        op_part = {}
        for h in range(H_PER):
            for b in range(B):
                g_t[h, b] = persist.tile([128, 4, TOK], BF16, tag=f"g{h}{b}",
                                         name=f"g{h}{b}")
        for b in range(B):
            op_part[b] = persist.tile([128, B * TOK // 128 // B, D],  # [128,2,D]
                                      BF16, tag=f"opart{b}",
                                      name=f"op_part{b}")
        wo_pack = [persist.tile([128, 4, D], BF16, tag=f"wop{h}",
                                name=f"wo_pack{h}") for h in range(H_PER)]
        bias_b = persist.tile([128, D], F32, tag="bias", name="bias_b")
        nc.scalar.dma_start(
            out=bias_b, in_=bo_t[0:1, :].to_broadcast([128, D]))

        def load_wo_pack(h, p):
            """wo_pack[h][t*64:(t+1)*64, p, :] = Wo rows of head 4p+h+2t."""
            for t in range(2):
                row = (4 * p + h + 2 * t) * DH
                st = wstage.tile([DH, D], F32, tag="wost", name="wost",
                                 bufs=2)
                nc.sync.dma_start(out=st, in_=wo_t[row:row + DH, :])
                nc.vector.tensor_copy(
                    out=wo_pack[h][t * DH:(t + 1) * DH, p, :], in_=st)

        def gather_g(h, b):
            # [8 slots, 64, TOK] -> [128, 4, TOK] with slot 2p+t at
            # partitions t*64:(t+1)*64, pair index p. Dispatched from the
            # gpsimd queue (waits on the A2A completion there, where nothing
            # critical queues behind it).
            nc.gpsimd.dma_start(
                out=g_t[h, b],
                in_=a2a_out[h, b].ap().rearrange("(pr t) p c -> (t p) pr c",
                                                 t=2))

        def gather_g11(half):
            nc.gpsimd.dma_start(
                out=g_t[1, 1][:, :, half * 128:(half + 1) * 128],
                in_=a2a_out11[half].ap().rearrange(
                    "(pr t) p c -> (t p) pr c", t=2))

        def mk_pass(h, b):
            """4 thunks; thunk (tb, dh) contracts the 4 packed pairs into
            psum for token block tb / out-column half dh. h==0 passes write
            bf16 partials (+bias); h==1 passes add the partials and DMA each
            finished 128-token block out as soon as both halves are done."""
            thunks = []
            ot = {}

            def mk(tb, dh):
                def run():
                    ps = ps_x.tile([128, 512], F32, tag="x", name="op_ps")
                    for p in range(4):
                        nc.tensor.matmul(
                            ps,
                            lhsT=g_t[h, b][:, p, tb * 128:(tb + 1) * 128],
                            rhs=wo_pack[h][:, p, dh * 512:(dh + 1) * 512],
                            start=(p == 0), stop=(p == 3))
                    if h == 0:
                        nc.vector.tensor_add(
                            op_part[b][:, tb, dh * 512:(dh + 1) * 512], ps,
                            bias_b[:, dh * 512:(dh + 1) * 512])
                    else:
                        if tb not in ot:
                            ot[tb] = opool.tile([128, D], F32, tag="ot",
                                                name=f"ot{b}{tb}")
                        nc.vector.tensor_add(
                            ot[tb][:, dh * 512:(dh + 1) * 512], ps,
                            op_part[b][:, tb, dh * 512:(dh + 1) * 512])
                        if dh == 1:
                            nc.scalar.dma_start(
                                out=out_t[b * TOK + tb * 128:
                                          b * TOK + (tb + 1) * 128, :],
                                in_=ot[tb])
                return run

            for tb in range(2):
                for dh in range(2):
                    thunks.append(mk(tb, dh))
            return thunks

        # ---- emission order ------------------------------------------------
        # weights first (small DMAs land in ~5us), then batch-0's transposes
        # INTERLEAVED with the projections that consume them. The PE queue is
        # strictly in-order, so emitting all 16 transpose blocks up front
        # head-of-line blocks the projection matmuls behind x DMAs that
        # haven't landed; interleaving keeps the PE dense from ~10us.
        xT0 = persist.tile([128, KT, S], BF16, tag="xT", name="xT0")
        qT0, kT0, vp0 = alloc_proj_tiles(0)
        ones_view0 = vp0.rearrange("p s (h c) -> p s h c",
                                   c=DH + 1)[:, :, :, DH:]
        nc.gpsimd.memset(ones_view0, 1.0)
        # first 4 x-blocks DMA first so the PE has transpose work from ~2us;
        # the weight stages land right behind them.
        for sb in range(4):
            transpose_x_step(0, xT0, sb)
        wq_bf, wk_bf, wv_bf = load_weights()
        for qb in range(4):
            for q in range(4):
                sb = qb * 4 + q
                if qb > 0:
                    transpose_x_step(0, xT0, sb)
                proj_v_step(xT0, wv_bf, vp0, sb)
            proj_qk_step(xT0, wq_bf, qT0, qb)
            proj_qk_step(xT0, wk_bf, kT0, qb)
            if qb == 0:
                # prewarm the ACT exp table set behind the first casts,
                # well before the first attention exp (~2.7us once)
                ewarm = const.tile([128, 16], F32)
                nc.vector.memset(ewarm, 0.0)
                ewarm_o = const.tile([128, 16], F32)
                nc.scalar.activation(ewarm_o, ewarm,
                                     mybir.ActivationFunctionType.Exp,
                                     scale=1.0)

        # batch 1's x-transpose + projections become PE filler inside the
        # first half of the attention pipeline (they keep PE at 100% duty
        # while ACT works through the exp chain); wo_pack loads ride along.
        xT1 = persist.tile([128, KT, S], BF16, tag="xT", name="xT1")
        qT1, kT1, vp1 = alloc_proj_tiles(1)
        ones_view1 = vp1.rearrange("p s (h c) -> p s h c",
                                   c=DH + 1)[:, :, :, DH:]
        nc.gpsimd.memset(ones_view1, 1.0)
        fillers = []
        for sb in range(SB):
            fillers.append(lambda sb=sb: transpose_x_step(1, xT1, sb))
        for w_bf, dest in ((wq_bf, qT1), (wk_bf, kT1)):
            for nt in range(NQS):
                fillers.append(
                    lambda w_bf=w_bf, dest=dest, nt=nt:
                        proj_qk_step(xT1, w_bf, dest, nt))
        for sb in range(SB):
            fillers.append(lambda sb=sb: proj_v_step(xT1, wv_bf, vp1, sb))
            if sb % 2 == 0:
                h, p = (sb // 2) % 2, sb // 4
                fillers.append(lambda h=h, p=p: load_wo_pack(h, p))

        # block order (h0,b0) (h1,b0) (h0,b1) (h1,b1): each block's A2A
        # triggers at its end (25/50/75/100%); its gather + out_proj pass
        # are scheduled one block later, when the exchange has landed.
        # steps per block: sum over q-span pairs qp of (lastB+1) = 8 + 16
        n_steps = sum(8 * qp + 8 for qp in range(NQS // 2))
        k0, k1, k2 = n_steps - 1, 2 * n_steps - 1, 3 * n_steps - 1
        pass00 = mk_pass(0, 0)
        pass10 = mk_pass(1, 0)
        # triggers are emitted BEFORE gathers on the gpsimd queue so a
        # gather's wait-for-collective never delays the next trigger; both
        # batch-0 gathers wait until the k2 boundary, after which the gpsimd
        # queue has no masking work left (block 4 masks on DVE), so their
        # collective waits can't stall the attention pipeline.
        pass01 = mk_pass(0, 1)
        pass11 = mk_pass(1, 1)
        actions = {
            k0: [lambda: exchange(0, 0)],
            k1: [lambda: exchange(1, 0)],
            k2: [lambda: exchange(0, 1), lambda: gather_g(0, 0),
                 lambda: gather_g(1, 0)],
            k2 + 2: [pass00[0]], k2 + 4: [pass00[1]],
            k2 + 6: [pass00[2]],
            # span 1 of (h1,b1) is staged by AV(k2+8): fire half A of the
            # final exchange immediately so it completes inside the pipeline.
            k2 + 8: [pass00[3], lambda: exchange11(0)],
            k2 + 10: [pass10[0]], k2 + 12: [pass10[1]],
            k2 + 14: [pass10[2]], k2 + 16: [pass10[3],
                                            lambda: gather_g(0, 1)],
            k2 + 20: [lambda: gather_g11(0)],
        }
        run_attention_pipeline(
            [
                attention_steps(0, 0, qT0, kT0, vp0),
                attention_steps(1, 0, qT0, kT0, vp0),
                attention_steps(0, 1, qT1, kT1, vp1),
                attention_steps(1, 1, qT1, kT1, vp1),
            ],
            fillers=fillers,
            actions=actions,
        )
        # tail: trigger half B of the final exchange; while it flies, the PE
        # runs the (h0,b1) pass and the half-A part of the (h1,b1) pass.
        # Only a 128KB exchange + one 4K-cycle pass trail everything.
        exchange11(1)
        for t in pass01:
            t()
        pass11[0]()
        pass11[1]()
        gather_g11(1)
        pass11[2]()
        pass11[3]()

    nc.compile()
    return nc


def shard_inputs(x, Wq, Wk, Wv, Wo, bo):
    """Full inputs -> per-core in_maps."""
    x = np.ascontiguousarray(np.asarray(x, dtype=np.float32))
    Wq = np.asarray(Wq, dtype=np.float32)
    Wk = np.asarray(Wk, dtype=np.float32)
    Wv = np.asarray(Wv, dtype=np.float32)
    Wo = np.ascontiguousarray(np.asarray(Wo, dtype=np.float32))
    bo = np.asarray(bo, dtype=np.float32).reshape(1, D)
    in_maps = []
    for c in range(N_CORES):
        cols = slice(c * DCOL, (c + 1) * DCOL)
        in_maps.append({
            "x": x,
            "wq": np.ascontiguousarray(Wq[:, cols]),
            "wk": np.ascontiguousarray(Wk[:, cols]),
            "wv": np.ascontiguousarray(Wv[:, cols]),
            "wo": Wo,
            "bo": bo,
        })
    return in_maps


def assemble_output(results):
    """Per-core out slices -> full [B, S, D]. Core c returns batch-0 tokens
    [256c, 256c+256) in rows 0:256; batch-1 tokens [128c, 128c+128) in rows
    256:384 and [1024+128c, 1024+128c+128) in rows 384:512 (the final
    exchange is split into two halves covering all 8 destinations each)."""
    out = np.empty((B, S, D), dtype=np.float32)
    for c in range(N_CORES):
        r = results[c]["out"]
        out[0, c * TOK:(c + 1) * TOK, :] = r[0:TOK]
        out[1, c * 128:(c + 1) * 128, :] = r[TOK:TOK + 128]
        out[1, 1024 + c * 128:1024 + (c + 1) * 128, :] = r[TOK + 128:2 * TOK]
    return out


def kernel(x, Wq, Wk, Wv, Wo, bo):
    if "nc" not in _CACHE:
        _CACHE["nc"] = build()
    nc = _CACHE["nc"]
    in_maps = shard_inputs(x, Wq, Wk, Wv, Wo, bo)
    res = run_bass_kernel_spmd(nc, in_maps, core_ids=list(range(N_CORES)))
    return assemble_output(res.results)


# revision 66
# speedup vs baseline: 1.2465x; 1.0142x over previous
"""Multi-head causal attention on 8 Trainium2 NeuronCores.

Problem: x [2, 2048, 1024] f32; Wq/Wk/Wv [1024, 1024]; Wo [1024, 1024]; bo [1024].
  q/k/v = split_heads(x @ W*)  (16 heads, head_dim 64)
  scores = q k^T, causal mask, / sqrt(1024), softmax, out = (w v) @ Wo + bo

Sharding: tensor-parallel over heads, 8-way (Megatron-style): core c computes
heads {2c, 2c+1} for BOTH batches. The concat+out_proj needs all heads, so
cores exchange attention outputs with 8-rank AllToAlls. Restructure vs the
296us baseline (whose tail after attention was ~75us: exposed A2As + a
zero-padded double out_proj at half clock) -- ~278us typical, best 263us:

 - per-(head,batch) AllToAlls triggered at the 25/50/75/100% marks of the
   attention pipeline (block order h0b0, h1b0, h0b1, h1b1). Triggered
   on-time and so 8-core-synchronized, each 256KB exchange completes in
   ~6us (vs ~30us when skewed). The LAST exchange (h1,b1) is further split
   into two 128KB halves (batch-1 tokens interleaved per-slot so each half
   covers all 8 destinations): half A + most out_proj work overlap the
   pipeline; only a ~6us exchange + one 4K-cycle pass trail it.
 - out_proj split into per-exchange partial passes, run as PE filler work
   inside the attention pipeline as each A2A lands.
 - out_proj contraction fully packed: the gathered 64-row head chunks are
   paired into 128-partition stationaries (chunk 2p in partitions 0:64,
   chunk 2p+1 in 64:128) against a row-gathered Wo ("wo_pack"), so out_proj
   costs 32768 PE cycles total instead of 65536 zero-padded ones.
 - output resharded: core j returns batch-0 tokens [256j,256j+256) and
   batch-1 tokens [128j,+128) and [1024+128j,+128); host reassembles.
 - A2A staging rides the sync queue (kept free of bulk loads: b1 x loads
   dispatch from the scalar queue), collective triggers ALWAYS precede
   gathers on the gpsimd queue, and the final block's causal masks run on
   DVE -- all so no collective-completion wait ever delays a trigger, a
   mask, or staging (each such coupling measured 30-70us).

Measured dead ends kept out: fp8 DoubleRow for proj/AV (this problem's
attention output is a near-uniform mean of ~random values, so the signal
averages down as fast as fp8 noise: 2-4% rel err vs the 2% budget); DMA
xbar transpose for x^T (serializes against collectives, destroying A2A
overlap); gpsimd casts (4x slower than DVE); --enable-ldw-opt (rejects
is_transpose/1-partition LDWEIGHTS).

On-chip layout trick (unchanged): attention is computed fully transposed
(scores^T = K Q^T in [k, q] layout) so the softmax weights come out exactly in
the layout the attn-value matmul wants as its moving operand, and the AV
result comes out as attn^T [d, q] which is exactly the stationary layout
out_proj wants. The softmax denominator is obtained for free by augmenting V
with a ones-column (row 64 of the AV psum accumulates sum(w)).

Compute dtype bf16 (fp32 accumulation in PSUM).
"""

from contextlib import ExitStack

import numpy as np

import concourse.bass as bass
import concourse.tile as tile
from concourse import bacc, mybir
from concourse.bass_utils import run_bass_kernel_spmd
from concourse.masks import make_identity

F32 = mybir.dt.float32
BF16 = mybir.dt.bfloat16

N_CORES = 8
B = 2
S = 2048
D = 1024
H = 16
DH = 64
H_PER = 2              # heads per core
DCOL = H_PER * DH      # 128: projection output cols per core
KT = D // 128          # 8 contraction tiles
SB = S // 128          # 16 sequence blocks
NQS = S // 512         # 4 q-spans
TOK = S // 8           # 256 output tokens per core per batch
SCALE = 1.0 / np.sqrt(np.float32(D))

_CACHE = {}


def build():
    """Build the SPMD program (identical on all 8 cores)."""
    nc = bacc.Bacc("TRN2", target_bir_lowering=False, debug=False)

    x_t = nc.dram_tensor("x", [B, S, D], F32, kind="ExternalInput")
    wq_t = nc.dram_tensor("wq", [D, DCOL], F32, kind="ExternalInput")
    wk_t = nc.dram_tensor("wk", [D, DCOL], F32, kind="ExternalInput")
    wv_t = nc.dram_tensor("wv", [D, DCOL], F32, kind="ExternalInput")
    wo_t = nc.dram_tensor("wo", [D, D], F32, kind="ExternalInput")
    bo_t = nc.dram_tensor("bo", [1, D], F32, kind="ExternalInput")
    # rows 0:TOK = batch-0 tokens [256c, 256c+256), rows TOK:2TOK = batch 1
    out_t = nc.dram_tensor("out", [B * TOK, D], F32, kind="ExternalOutput")

    # collective buffers (internal DRAM): one A2A per (local head, batch).
    # slot j of a2a_in goes to rank j = that head's attn^T restricted to
    # tokens [256j, 256j+256) of that batch; slot c of a2a_out arrived from
    # rank c = global head 2c+h for MY 256-token slice.
    warm_in = nc.dram_tensor("warm_in", [8, 16], F32)
    warm_out = nc.dram_tensor("warm_out", [8, 16], F32)
    a2a_in = {}
    a2a_out = {}
    for h in range(H_PER):
        for b in range(B):
            a2a_in[h, b] = nc.dram_tensor(f"a2a_in{h}{b}", [8, DH, TOK], BF16)
            a2a_out[h, b] = nc.dram_tensor(f"a2a_out{h}{b}", [8, DH, TOK], BF16)
    # the LAST exchange (h1, b1) is split into two 128KB halves so the first
    # half (+ its out_proj work) overlaps the pipeline tail; batch-1 tokens
    # are interleaved per-slot (slot j: tokens [128j,128j+128) of each
    # 1024-token half) so each half covers all 8 destinations.
    a2a_in11 = [nc.dram_tensor(f"a2a_in11{x}", [8, DH, 128], BF16)
                for x in "ab"]
    a2a_out11 = [nc.dram_tensor(f"a2a_out11{x}", [8, DH, 128], BF16)
                 for x in "ab"]

    with tile.TileContext(nc) as tc, ExitStack() as ctx:
        const = ctx.enter_context(tc.tile_pool(name="const", bufs=1))
        persist = ctx.enter_context(tc.tile_pool(name="persist", bufs=1))
        stage = ctx.enter_context(tc.tile_pool(name="stage", bufs=2))
        wstage = ctx.enter_context(tc.tile_pool(name="wstage", bufs=1))
        wpool = ctx.enter_context(tc.tile_pool(name="wpool", bufs=6))
        rbpool = ctx.enter_context(tc.tile_pool(name="rbpool", bufs=2))
        spool = ctx.enter_context(tc.tile_pool(name="spool", bufs=2))
        opool = ctx.enter_context(tc.tile_pool(name="opool", bufs=2))
        ps_mm = ctx.enter_context(tc.tile_pool(name="ps_mm", bufs=2, space="PSUM"))
        ps_o = ctx.enter_context(tc.tile_pool(name="ps_o", bufs=2, space="PSUM"))
        ps_x = ctx.enter_context(tc.tile_pool(name="ps_x", bufs=2, space="PSUM"))

        # ---- warmup collective: absorbs the per-execution ncfw entry cost
        # concurrently with the compute phase.
        nc.gpsimd.collective_compute(
            "AllToAll", mybir.AluOpType.bypass,
            replica_groups=[list(range(8))],
            ins=[warm_in.ap().opt()], outs=[warm_out.ap().opt()],
        )

        identity = const.tile([128, 128], BF16)
        make_identity(nc, identity)
        ones1 = const.tile([1, DH], BF16)
        nc.vector.memset(ones1, 1.0)
        # 4 causal masks (mask[r][p,f] = 1 iff f - p - 128r >= 0) for the
        # final attention block, whose masking runs on DVE instead of gpsimd
        # so the gpsimd queue (collective triggers + gathers) is never a
        # masking dependency at the tail.
        masks = []
        for r in range(4):
            mk = const.tile([128, 512], BF16, name=f"mask{r}")
            nc.vector.memset(mk, 1.0)
            nc.gpsimd.affine_select(
                out=mk, in_=mk,
                pattern=[[1, 512]],
                compare_op=mybir.AluOpType.is_ge,
                fill=0.0,
                base=-128 * r,
                channel_multiplier=-1)
            masks.append(mk)

        attnT = [
            persist.tile([128, S], BF16, tag=f"attnT{b}", name=f"attnT{b}")
            for b in range(B)
        ]

        # ---- phase helpers -------------------------------------------------
        def transpose_x_step(b, xT, sb):
            """One 128-row block of x[b] -> x^T bf16 columns, via PE
            transpose. Batch 0's cast runs on the otherwise-idle ACT."""
            xn = stage.tile([128, D], F32, tag="xn", name="xn", bufs=4)
            # b1's bulk loads dispatch from the scalar queue so the sync
            # queue (A2A staging) never backs up behind 8MB of x traffic;
            # their buffer-free waits resolve via DVE casts, which never
            # depend on later scalar-queue work (no cross-queue cycle).
            (nc.sync if b == 0 else nc.scalar).dma_start(
                out=xn, in_=x_t[b, sb * 128:(sb + 1) * 128, :])
            xb = stage.tile([128, D], BF16, tag=f"xb{b}", name="xb", bufs=2)
            # b0's cast runs on the otherwise-idle ACT; b1's on DVE (gpsimd
            # casts measured 4x slower, and ACT is exp-critical by then).
            if b == 0:
                nc.scalar.copy(xb, xn)
            else:
                nc.vector.tensor_copy(out=xb, in_=xn)
            for g in range(2):  # 2 groups of 4 d-blocks -> one psum bank
                pt = ps_x.tile([128, 4, 128], BF16, tag="x", name="pt")
                for k in range(4):
                    kt = g * 4 + k
                    nc.tensor.transpose(
                        pt[:, k, :], xb[:, kt * 128:(kt + 1) * 128], identity)
                dst = xT[:, g * 4:(g + 1) * 4, sb * 128:(sb + 1) * 128]
                nc.vector.tensor_copy(out=dst, in_=pt)

        def load_weights():
            def load_cast(dram_ap, kt_cols, name):
                st = wstage.tile([128, KT, kt_cols], F32, tag="wst",
                                 name="wst")
                nc.sync.dma_start(
                    out=st, in_=dram_ap.rearrange("(kt p) c -> p kt c", p=128))
                bf = persist.tile([128, KT, kt_cols], BF16, tag=name, name=name)
                nc.vector.tensor_copy(out=bf, in_=st)
                return bf

            wq_bf = load_cast(wq_t[:, :], DCOL, "wq")
            wk_bf = load_cast(wk_t[:, :], DCOL, "wk")
            wv_bf = load_cast(wv_t[:, :], DCOL, "wv")
            return wq_bf, wk_bf, wv_bf

        def alloc_proj_tiles(b):
            # Q^T / K^T are stored per-head, zero-padded to 128 partitions
            # (rows 64:128 = 0) because matmuls with contraction dim <= 64
            # run at HALF rate on the PE.
            qT = [persist.tile([128, S], BF16, tag=f"qT{b}{h}",
                               name=f"qT{b}{h}") for h in range(H_PER)]
            kTt = [persist.tile([128, S], BF16, tag=f"kT{b}{h}",
                                name=f"kT{b}{h}") for h in range(H_PER)]
            vp = persist.tile([128, SB, H_PER * (DH + 1)], BF16,
                              tag=f"vp{b}", name=f"vp{b}")
            for t in qT + kTt:
                nc.gpsimd.memset(t[DH:128, :], 0.0)
            return qT, kTt, vp

        def proj_qk_step(xT, w_bf, dest, nt):
            ps = ps_x.tile([128, 512], F32, tag="x", name="ps")
            for kt in range(KT):
                nc.tensor.matmul(
                    ps, lhsT=w_bf[:, kt, :],
                    rhs=xT[:, kt, nt * 512:(nt + 1) * 512],
                    start=(kt == 0), stop=(kt == KT - 1))
            for h in range(H_PER):
                nc.vector.tensor_copy(
                    out=dest[h][0:DH, nt * 512:(nt + 1) * 512],
                    in_=ps[h * DH:(h + 1) * DH, :])

        def proj_v_step(xT, wv_bf, vp, sb):
            ps = ps_x.tile([128, 512], F32, tag="x", name="ps")
            for kt in range(KT):
                nc.tensor.matmul(
                    ps[:, 0:DCOL], lhsT=xT[:, kt, sb * 128:(sb + 1) * 128],
                    rhs=wv_bf[:, kt, :],
                    start=(kt == 0), stop=(kt == KT - 1))
            dst = vp.rearrange("p s (h c) -> p s h c", c=DH + 1)[:, sb, :, :DH]
            nc.vector.tensor_copy(
                out=dst, in_=ps[:, 0:DCOL].rearrange("p (h c) -> p h c", c=DH))

        # Attention, software-pipelined GLOBALLY across (batch, head, q-span)
        # in PAIRS of k-blocks: scores for a pair land in a 2-bank psum tile,
        # ONE exp covers both halves, and the AV matmuls trail DEPTH pairs
        # behind, so the PE stream has a single priming point for the whole
        # attention phase.
        DEPTH = 4

        def normalize_evict(b, h, qs, o_ps):
            hr = h * DH
            denom = spool.tile([1, 512], F32, tag="den", name="denom")
            nc.vector.tensor_copy(out=denom, in_=o_ps[DH:DH + 1, :])
            recip_f = spool.tile([1, 512], F32, tag="recf", name="recip_f")
            nc.vector.reciprocal_approx_fast(out=recip_f, in_=denom)
            recip = spool.tile([1, 512], BF16, tag="rec", name="recip")
            nc.vector.tensor_copy(out=recip, in_=recip_f)
            # broadcast recip across the 64 partitions through the PE array
            # (the only cross-partition fabric that isn't a DMA round-trip).
            rb_ps = ps_x.tile([DH, 512], F32, tag="x", name="rb_ps")
            nc.tensor.matmul(rb_ps, lhsT=ones1, rhs=recip,
                             start=True, stop=True)
            rb = rbpool.tile([DH, 512], F32, tag="rb", name="rb")
            nc.vector.tensor_copy(out=rb, in_=rb_ps)
            dst = attnT[b][hr:hr + DH, qs * 512:(qs + 1) * 512]
            nc.vector.tensor_mul(dst, o_ps[0:DH, :], rb)
            # stage this span's attn^T to the A2A input right away. b0:
            # slot j = contiguous 256-token slice j (span covers slots 2qs,
            # 2qs+1). b1: interleaved map -- token t<1024 goes to slot
            # t//128 col t%128 (half A), t>=1024 to slot (t-1024)//128 col
            # 128+ (half B) -- so each half covers all 8 destinations.
            if b == 0:
                for j in range(2):
                    nc.sync.dma_start(
                        out=a2a_in[h, b][2 * qs + j],
                        in_=dst[:, j * TOK:(j + 1) * TOK])
            else:
                half, base = qs // 2, 4 * (qs % 2)
                for i in range(4):
                    if h == 0:
                        out_ap = a2a_in[0, 1][base + i, :,
                                              half * 128:(half + 1) * 128]
                    else:
                        out_ap = a2a_in11[half][base + i]
                    nc.sync.dma_start(
                        out=out_ap, in_=dst[:, i * 128:(i + 1) * 128])

        def attention_steps(h, b, qT, kTt, vp):
            """Yield (emit_scores, emit_av) closures, one pair per k-block,
            processing TWO q-spans at once (same stationary operand ->
            LDWEIGHTS dedups between the two matmuls). Epilogues fire from
            the AV closure that completes each span."""
            qTh, kTh = qT[h], kTt[h]
            vslice = vp[:, :, h * (DH + 1):(h + 1) * (DH + 1)]
            for qp in range(NQS // 2):
                qsA, qsB = 2 * qp, 2 * qp + 1
                lastA, lastB = 4 * qsA + 3, 4 * qsB + 3
                o_A = ps_o.tile([DH + 1, 512], F32, tag="o", name="o_A")
                o_B = ps_o.tile([DH + 1, 512], F32, tag="o", name="o_B")
                box = {}

                def mk_scores(kb, qsA=qsA, qsB=qsB, lastA=lastA, box=box):
                    def emit_scores():
                        s_ps = ps_mm.tile([128, 2, 512], F32, tag="mm",
                                          name="s_ps")
                        spans = ([(0, qsA)] if kb <= lastA else []) + [(1, qsB)]
                        for i, qs in spans:
                            nc.tensor.matmul(
                                s_ps[:, i, :],
                                lhsT=kTh[:, kb * 128:(kb + 1) * 128],
                                rhs=qTh[:, qs * 512:(qs + 1) * 512],
                                start=True, stop=True)
                        w_bf_t = wpool.tile([128, 2, 512], BF16, tag="w",
                                            name="w_bf_t")
                        if kb < 4 * qsA and len(spans) == 2:
                            # clean interior for both spans: one big exp
                            nc.scalar.activation(
                                w_bf_t, s_ps, mybir.ActivationFunctionType.Exp,
                                scale=float(SCALE))
                        else:
                            for i, qs in spans:
                                lo = max(0, 128 * (kb - 4 * qs))
                                nc.scalar.activation(
                                    w_bf_t[:, i, lo:512], s_ps[:, i, lo:512],
                                    mybir.ActivationFunctionType.Exp,
                                    scale=float(SCALE))
                        for i, qs in spans:
                            if kb >= 4 * qs:
                                # causal: keep iff (512qs+f) - (128kb+p) >= 0.
                                # Final block masks on DVE so the gpsimd
                                # queue (gathers + triggers) is never a
                                # masking dependency; the stale [0:lo] region
                                # is finite (prior exp outputs) so mul-by-0
                                # safely zeroes it.
                                if h == 1 and b == 1:
                                    nc.vector.tensor_mul(
                                        w_bf_t[:, i, :], w_bf_t[:, i, :],
                                        masks[kb - 4 * qs])
                                else:
                                    nc.gpsimd.affine_select(
                                        out=w_bf_t[:, i, :],
                                        in_=w_bf_t[:, i, :],
                                        pattern=[[1, 512]],
                                        compare_op=mybir.AluOpType.is_ge,
                                        fill=0.0,
                                        base=512 * qs - 128 * kb,
                                        channel_multiplier=-1)
                        box[kb] = w_bf_t
                    return emit_scores

                def mk_av(kb, qsA=qsA, qsB=qsB, lastA=lastA, lastB=lastB,
                          o_A=o_A, o_B=o_B, box=box):
                    def emit_av():
                        if kb <= lastA:
                            nc.tensor.matmul(
                                o_A, lhsT=vslice[:, kb, :],
                                rhs=box[kb][:, 0, :],
                                start=(kb == 0), stop=(kb == lastA))
                        nc.tensor.matmul(
                            o_B, lhsT=vslice[:, kb, :],
                            rhs=box[kb][:, 1, :],
                            start=(kb == 0), stop=(kb == lastB))
                        del box[kb]
                        if kb == lastA:
                            normalize_evict(b, h, qsA, o_A)
                        if kb == lastB:
                            normalize_evict(b, h, qsB, o_B)
                    return emit_av

                for kb in range(lastB + 1):
                    yield mk_scores(kb), mk_av(kb)

        def run_attention_pipeline(blocks, fillers=(), actions=None):
            """blocks: list of generators from attention_steps. Runs one
            DEPTH-deep pipeline across all of them. Fillers (extra PE work)
            are injected one-per-step and must all be emitted before the
            third block starts (its inputs come from the fillers). actions
            maps a global step index k -> thunks emitted right after step
            k's AV (used for collective triggers, gathers, out_proj passes).
            """
            steps = []
            bounds = []
            for blk in blocks:
                blksteps = list(blk)
                steps.extend(blksteps)
                bounds.append(len(steps))
            n = len(steps)
            acts = actions or {}
            fillers = list(fillers)
            fi = 0
            # fillers spread proportionally across the first two blocks
            # (their outputs feed block 3); even spreading keeps every block
            # short so the A2A triggers stay evenly spaced.
            span = max(1, bounds[1] - DEPTH - 2)
            for i in range(n + DEPTH):
                if i < n:
                    steps[i][0]()          # scores/exp/mask for step i
                want = min(len(fillers), ((i + 1) * len(fillers)) // span)
                while fi < want:
                    fillers[fi]()
                    fi += 1
                if i >= DEPTH:
                    k = i - DEPTH
                    steps[k][1]()          # AV for step k
                    for t in acts.get(k, ()):
                        t()
            while fi < len(fillers):
                fillers[fi]()
                fi += 1
            return bounds

        def exchange(h, b):
            """Trigger the (h, b) A2A (inputs already staged per-span)."""
            nc.gpsimd.collective_compute(
                "AllToAll", mybir.AluOpType.bypass,
                replica_groups=[list(range(8))],
                ins=[a2a_in[h, b].ap().opt()], outs=[a2a_out[h, b].ap().opt()],
            )

        def exchange11(half):
            """Trigger one 128KB half of the final (h1, b1) exchange."""
            nc.gpsimd.collective_compute(
                "AllToAll", mybir.AluOpType.bypass,
                replica_groups=[list(range(8))],
                ins=[a2a_in11[half].ap().opt()],
                outs=[a2a_out11[half].ap().opt()],
            )

        # ---- out_proj machinery -------------------------------------------
        # After the (h, b) A2A, slot c holds global head 2c+h's attn^T for my
        # 256 tokens of batch b. Slots are PAIRED into 128-partition tiles
        # (slot 2p in partitions 0:64, slot 2p+1 in 64:128), contracted
        # against wo_pack[h] whose rows are gathered the same way, and
        # accumulated over the 4 pairs -- a fully packed contraction.
        g_t = {}
        op_part = {}
        for h in range(H_PER):
            for b in range(B):
                g_t[h, b] = persist.tile([128, 4, TOK], BF16, tag=f"g{h}{b}",
                                         name=f"g{h}{b}")
        for b in range(B):
            op_part[b] = persist.tile([128, B * TOK // 128 // B, D],  # [128,2,D]
                                      BF16, tag=f"opart{b}",
                                      name=f"op_part{b}")
        wo_pack = [persist.tile([128, 4, D], BF16, tag=f"wop{h}",
                                name=f"wo_pack{h}") for h in range(H_PER)]
        bias_b = persist.tile([128, D], F32, tag="bias", name="bias_b")
        nc.scalar.dma_start(
            out=bias_b, in_=bo_t[0:1, :].to_broadcast([128, D]))

        def load_wo_pack(h, p):
            """wo_pack[h][t*64:(t+1)*64, p, :] = Wo rows of head 4p+h+2t."""
            for t in range(2):
                row = (4 * p + h + 2 * t) * DH
                st = wstage.tile([DH, D], F32, tag="wost", name="wost",
                                 bufs=2)
                nc.sync.dma_start(out=st, in_=wo_t[row:row + DH, :])
                nc.vector.tensor_copy(
                    out=wo_pack[h][t * DH:(t + 1) * DH, p, :], in_=st)

        def gather_g(h, b):
            # [8 slots, 64, TOK] -> [128, 4, TOK] with slot 2p+t at
            # partitions t*64:(t+1)*64, pair index p. Dispatched from the
            # gpsimd queue (waits on the A2A completion there, where nothing
            # critical queues behind it).
            nc.gpsimd.dma_start(
                out=g_t[h, b],
                in_=a2a_out[h, b].ap().rearrange("(pr t) p c -> (t p) pr c",
                                                 t=2))

        def gather_g11(half):
            nc.gpsimd.dma_start(
                out=g_t[1, 1][:, :, half * 128:(half + 1) * 128],
                in_=a2a_out11[half].ap().rearrange(
                    "(pr t) p c -> (t p) pr c", t=2))

        def mk_pass(h, b):
            """4 thunks; thunk (tb, dh) contracts the 4 packed pairs into
            psum for token block tb / out-column half dh. h==0 passes write
            bf16 partials (+bias); h==1 passes add the partials and DMA each
            finished 128-token block out as soon as both halves are done."""
            thunks = []
            ot = {}

            def mk(tb, dh):
                def run():
                    ps = ps_x.tile([128, 512], F32, tag="x", name="op_ps")
                    for p in range(4):
                        nc.tensor.matmul(
                            ps,
                            lhsT=g_t[h, b][:, p, tb * 128:(tb + 1) * 128],
                            rhs=wo_pack[h][:, p, dh * 512:(dh + 1) * 512],
                            start=(p == 0), stop=(p == 3))
                    if h == 0:
                        nc.vector.tensor_add(
                            op_part[b][:, tb, dh * 512:(dh + 1) * 512], ps,
                            bias_b[:, dh * 512:(dh + 1) * 512])
                    else:
                        if tb not in ot:
                            ot[tb] = opool.tile([128, D], F32, tag="ot",
                                                name=f"ot{b}{tb}")
                        nc.vector.tensor_add(
                            ot[tb][:, dh * 512:(dh + 1) * 512], ps,
                            op_part[b][:, tb, dh * 512:(dh + 1) * 512])
                        if dh == 1:
                            nc.scalar.dma_start(
                                out=out_t[b * TOK + tb * 128:
                                          b * TOK + (tb + 1) * 128, :],
                                in_=ot[tb])
                return run

            for tb in range(2):
                for dh in range(2):
                    thunks.append(mk(tb, dh))
            return thunks

        # ---- emission order ------------------------------------------------
        # weights first (small DMAs land in ~5us), then batch-0's transposes
        # INTERLEAVED with the projections that consume them. The PE queue is
        # strictly in-order, so emitting all 16 transpose blocks up front
        # head-of-line blocks the projection matmuls behind x DMAs that
        # haven't landed; interleaving keeps the PE dense from ~10us.
        xT0 = persist.tile([128, KT, S], BF16, tag="xT", name="xT0")
        qT0, kT0, vp0 = alloc_proj_tiles(0)
        ones_view0 = vp0.rearrange("p s (h c) -> p s h c",
                                   c=DH + 1)[:, :, :, DH:]
        nc.gpsimd.memset(ones_view0, 1.0)
        # first 4 x-blocks DMA first so the PE has transpose work from ~2us;
        # the weight stages land right behind them.
        for sb in range(4):
            transpose_x_step(0, xT0, sb)
        wq_bf, wk_bf, wv_bf = load_weights()
        for qb in range(4):
            for q in range(4):
                sb = qb * 4 + q
                if qb > 0:
                    transpose_x_step(0, xT0, sb)
                proj_v_step(xT0, wv_bf, vp0, sb)
            proj_qk_step(xT0, wq_bf, qT0, qb)
            proj_qk_step(xT0, wk_bf, kT0, qb)
            if qb == 0:
                # prewarm the ACT exp table set behind the first casts,
                # well before the first attention exp (~2.7us once)
                ewarm = const.tile([128, 16], F32)
                nc.vector.memset(ewarm, 0.0)
                ewarm_o = const.tile([128, 16], F32)
                nc.scalar.activation(ewarm_o, ewarm,
                                     mybir.ActivationFunctionType.Exp,
                                     scale=1.0)

        # batch 1's x-transpose + projections become PE filler inside the
        # first half of the attention pipeline (they keep PE at 100% duty
        # while ACT works through the exp chain); wo_pack loads ride along.
        xT1 = persist.tile([128, KT, S], BF16, tag="xT", name="xT1")
        qT1, kT1, vp1 = alloc_proj_tiles(1)
        ones_view1 = vp1.rearrange("p s (h c) -> p s h c",
                                   c=DH + 1)[:, :, :, DH:]
        nc.gpsimd.memset(ones_view1, 1.0)
        fillers = []
        for sb in range(SB):
            fillers.append(lambda sb=sb: transpose_x_step(1, xT1, sb))
        for w_bf, dest in ((wq_bf, qT1), (wk_bf, kT1)):
            for nt in range(NQS):
                fillers.append(
                    lambda w_bf=w_bf, dest=dest, nt=nt:
                        proj_qk_step(xT1, w_bf, dest, nt))
        for sb in range(SB):
            fillers.append(lambda sb=sb: proj_v_step(xT1, wv_bf, vp1, sb))
            if sb % 2 == 0:
                h, p = (sb // 2) % 2, sb // 4
                fillers.append(lambda h=h, p=p: load_wo_pack(h, p))

        # block order (h0,b0) (h1,b0) (h0,b1) (h1,b1): each block's A2A
        # triggers at its end (25/50/75/100%); its gather + out_proj pass
        # are scheduled one block later, when the exchange has landed.
        # steps per block: sum over q-span pairs qp of (lastB+1) = 8 + 16
        n_steps = sum(8 * qp + 8 for qp in range(NQS // 2))
        k0, k1, k2 = n_steps - 1, 2 * n_steps - 1, 3 * n_steps - 1
        pass00 = mk_pass(0, 0)
        pass10 = mk_pass(1, 0)
        # triggers are emitted BEFORE gathers on the gpsimd queue so a
        # gather's wait-for-collective never delays the next trigger; both
        # batch-0 gathers wait until the k2 boundary, after which the gpsimd
        # queue has no masking work left (block 4 masks on DVE), so their
        # collective waits can't stall the attention pipeline.
        pass01 = mk_pass(0, 1)
        pass11 = mk_pass(1, 1)
        actions = {
            k0: [lambda: exchange(0, 0)],
            k1: [lambda: exchange(1, 0)],
            k2: [lambda: exchange(0, 1), lambda: gather_g(0, 0),
                 lambda: gather_g(1, 0)],
            k2 + 2: [pass00[0]], k2 + 4: [pass00[1]],
            k2 + 6: [pass00[2]],
            # span 1 of (h1,b1) is staged by AV(k2+8): fire half A of the
            # final exchange immediately so it completes inside the pipeline.
            k2 + 8: [pass00[3], lambda: exchange11(0)],
            k2 + 10: [pass10[0]], k2 + 12: [pass10[1]],
            k2 + 14: [pass10[2]], k2 + 16: [pass10[3],
                                            lambda: gather_g(0, 1)],
            k2 + 20: [lambda: gather_g11(0)],
        }
        run_attention_pipeline(
            [
                attention_steps(0, 0, qT0, kT0, vp0),
                attention_steps(1, 0, qT0, kT0, vp0),
                attention_steps(0, 1, qT1, kT1, vp1),
                attention_steps(1, 1, qT1, kT1, vp1),
            ],
            fillers=fillers,
            actions=actions,
        )
        # tail: trigger half B of the final exchange; while it flies, the PE
        # runs the (h0,b1) pass and the half-A part of the (h1,b1) pass.
        # Only a 128KB exchange + one 4K-cycle pass trail everything.
        exchange11(1)
        for t in pass01:
            t()
        pass11[0]()
        pass11[1]()
        gather_g11(1)
        pass11[2]()
        pass11[3]()

    nc.compile()
    return nc


def shard_inputs(x, Wq, Wk, Wv, Wo, bo):
    """Full inputs -> per-core in_maps."""
    x = np.ascontiguousarray(np.asarray(x, dtype=np.float32))
    Wq = np.asarray(Wq, dtype=np.float32)
    Wk = np.asarray(Wk, dtype=np.float32)
    Wv = np.asarray(Wv, dtype=np.float32)
    Wo = np.ascontiguousarray(np.asarray(Wo, dtype=np.float32))
    bo = np.asarray(bo, dtype=np.float32).reshape(1, D)
    in_maps = []
    for c in range(N_CORES):
        cols = slice(c * DCOL, (c + 1) * DCOL)
        in_maps.append({
            "x": x,
            "wq": np.ascontiguousarray(Wq[:, cols]),
            "wk": np.ascontiguousarray(Wk[:, cols]),
            "wv": np.ascontiguousarray(Wv[:, cols]),
            "wo": Wo,
            "bo": bo,
        })
    return in_maps


def assemble_output(results):
    """Per-core out slices -> full [B, S, D]. Core c returns batch-0 tokens
    [256c, 256c+256) in rows 0:256; batch-1 tokens [128c, 128c+128) in rows
    256:384 and [1024+128c, 1024+128c+128) in rows 384:512 (the final
    exchange is split into two halves covering all 8 destinations each)."""
    out = np.empty((B, S, D), dtype=np.float32)
    for c in range(N_CORES):
        r = results[c]["out"]
        out[0, c * TOK:(c + 1) * TOK, :] = r[0:TOK]
        out[1, c * 128:(c + 1) * 128, :] = r[TOK:TOK + 128]
        out[1, 1024 + c * 128:1024 + (c + 1) * 128, :] = r[TOK + 128:2 * TOK]
    return out


def kernel(x, Wq, Wk, Wv, Wo, bo):
    if "nc" not in _CACHE:
        _CACHE["nc"] = build()
    nc = _CACHE["nc"]
    in_maps = shard_inputs(x, Wq, Wk, Wv, Wo, bo)
    res = run_bass_kernel_spmd(nc, in_maps, core_ids=list(range(N_CORES)))
    return assemble_output(res.results)
